# revision 60
# baseline (speedup 1.0000x reference)
"""Multi-head attention (B=2, S=2048, D=1024, H=16) on 8 TRN2 NeuronCores.

Sharding: batch (2) x head-groups (4 heads/core). Each core computes its
batch's QKV projections for its 4 heads, causal attention, and a partial
output projection over its head slice; the host sums the 4 partials per
batch and adds the output bias.

Layout strategy: everything runs in "transposed" orientation so no on-chip
transposes are needed:
  q2^T[dm, s] = Wq[dm,:] @ Q^T       (host supplies Q^T and Wq^T)
  scores^T[j, si] = k^T.T @ q^T      (d_h contraction, 2 heads row-tiled)
  attn^T = exp(scores^T/8 + mask)    (mask added pre-exp via -60000*tri matmul)
  ctx^T+denom = [v | 1].T @ attn^T   (ones column gives softmax denominator)
  out[s, n] = ctxn^T.T @ Wo^T        (K=128 chunks x2)
v2: single interleaved pipeline -- QKV projection and output projection
units are emitted as fillers inside the attention jt loop so the tensor
queue never drains (keeps HAM at full clock); mask applied pre-exp on the
tensor engine; straddle exp in one strided-AP instruction; f16 output.
"""

import numpy as np

B, S, D, H, DH = 2, 2048, 1024, 16, 64
NCORES = 8
CORES_PER_BATCH = 4
HPC = H // CORES_PER_BATCH  # heads per core = 4
NEG = -60000.0  # exp((x+NEG)/8) == 0 exactly in fp32; fits in fp16
MMDT = "f16"

TRACE = False  # test.py sets True to collect an NTFF profile
LAST_RESULT = None  # BassKernelResults from the last run (for test.py)

_built = {}


def _build_v2(mmdt: str):
    key = ("v2", mmdt)
    if key in _built:
        return _built[key]
    import concourse.mybir as mybir
    import concourse.tile as tile
    from concourse import bacc
    from concourse.bass import ts, ds

    f32 = mybir.dt.float32
    DT = {"f32r": mybir.dt.float32r, "f16": mybir.dt.float16,
          "bf16": mybir.dt.bfloat16}[mmdt]
    DTNP = {"f32r": f32, "f16": mybir.dt.float16, "bf16": mybir.dt.bfloat16}[mmdt]
    EXP = mybir.ActivationFunctionType.Exp
    DT8 = mybir.dt.float8e4
    DROW = mybir.MatmulPerfMode.DoubleRow

    nc = bacc.Bacc("TRN2")
    qt = nc.dram_tensor("qt", [D, S], DT8, kind="ExternalInput")
    kt = nc.dram_tensor("kt", [D, S], DT8, kind="ExternalInput")
    # V split by key range: keys < 512 stay f16 (feed the exact early-query
    # path); keys >= 512 are fp8 (only ever read by spread-attention queries)
    vt16 = nc.dram_tensor("vt16", [D, 512], DTNP, kind="ExternalInput")
    vt8 = nc.dram_tensor("vt8", [D, S - 512], DT8, kind="ExternalInput")
    wq = nc.dram_tensor("wq", [D, HPC * DH], DT8, kind="ExternalInput")
    wk = nc.dram_tensor("wk", [D, HPC * DH], DT8, kind="ExternalInput")
    wv = nc.dram_tensor("wv", [D, HPC * DH], DTNP, kind="ExternalInput")
    wv8 = nc.dram_tensor("wv8", [D, HPC * DH], DT8, kind="ExternalInput")
    wo = nc.dram_tensor("wo", [2, 128, D], DT8, kind="ExternalInput")
    wo16 = nc.dram_tensor("wo16", [2, 128, D], DTNP, kind="ExternalInput")
    bq = nc.dram_tensor("bq", [128, 2], f32, kind="ExternalInput")
    bk = nc.dram_tensor("bk", [128, 2], f32, kind="ExternalInput")
    bv = nc.dram_tensor("bv", [1, HPC * DH], DTNP, kind="ExternalInput")
    mtri = nc.dram_tensor("mtri", [128, 2, 128], DT8, kind="ExternalInput")
    out = nc.dram_tensor("out", [S, D], mybir.dt.float16, kind="ExternalOutput")

    NSB = S // 512   # 4 si-blocks of 512
    NST = S // 128   # 16 s-tiles / j-tiles of 128

    import contextlib
    with tile.TileContext(nc) as tc, contextlib.ExitStack() as cx:
        pp = cx.enter_context(tc.tile_pool(name="persist", bufs=1))
        sc_ps = cx.enter_context(tc.tile_pool(name="sc_ps", bufs=2, space="PSUM"))
        ctx_ps = cx.enter_context(tc.tile_pool(name="ctx_ps", bufs=1, space="PSUM"))
        mm_ps = cx.enter_context(tc.tile_pool(name="mm_ps", bufs=2, space="PSUM"))
        sp = cx.enter_context(tc.tile_pool(name="stream", bufs=1))
        ap = cx.enter_context(tc.tile_pool(name="attn", bufs=4))
        smp = cx.enter_context(tc.tile_pool(name="small", bufs=3))
        op = cx.enter_context(tc.tile_pool(name="outp", bufs=4))

        # ---- persistent tiles + constant DMAs (small, on scalar queue) ----
        bq_t = pp.tile([128, 2], f32)
        nc.scalar.dma_start(out=bq_t, in_=bq[:, :])
        bk_t = pp.tile([128, 2], f32)
        nc.scalar.dma_start(out=bk_t, in_=bk[:, :])
        bv_t = pp.tile([1, HPC * DH], DT)
        nc.scalar.dma_start(out=bv_t, in_=bv[:, :].bitcast(DT))
        # 0/1 causal keep-mask for the diagonal blocks, applied post-exp on
        # the pool engine (attn tiles are SBUF, which gpsimd can reach --
        # this keeps the mask off the busy DVE/ACT PSUM path entirely)
        mtri_t = pp.tile([128, 2, 128], DT8)
        nc.scalar.dma_start(out=mtri_t, in_=mtri[:, :, :])

        ones_c = pp.tile([1, 128], DT)
        nc.vector.memset(ones_c.bitcast(DTNP), 1.0)
        warm = pp.tile([128, 512], DT, name="warm")
        nc.vector.memset(warm.bitcast(DTNP), 1.0)

        q2t = [pp.tile([128, S], DT, tag=f"q2t{i}", name=f"q2t{i}") for i in range(2)]
        k2t = [pp.tile([128, S], DT, tag=f"k2t{i}", name=f"k2t{i}") for i in range(2)]
        # fp8 vaug in 96-wide head blocks (v 0:64 | ones 64 | zero pad 65:96):
        # dual-fp8 ldweights requires a 32-multiple stationary free size
        HB = 96
        vaug = pp.tile([128, NST, HPC * HB], DT8)
        for h in range(HPC):
            nc.vector.memset(vaug[:, :, ds(h * HB + 64, 1)], 1.0)
            nc.vector.memset(vaug[:, :, ds(h * HB + 65, HB - 65)], 0.0)
        # f16 copies for the first query block (sb=0): near-one-hot attention
        # there makes ctx ~ a raw V row, so fp8 V/ctx/Wo would be ~4% off
        vaug16 = pp.tile([128, 4, HPC * (DH + 1)], DT, name="vaug16")
        for h in range(HPC):
            nc.vector.memset(vaug16[:, :, ds(h * 65 + 64, 1)].bitcast(DTNP), 1.0)
        ctxt = pp.tile([128, 2, S], DT8, name="ctxt")
        ctxt16 = pp.tile([128, 2, 512], DT, name="ctxt16")

        wq_t = pp.tile([128, 8, 256], DT8)
        wk_t = pp.tile([128, 8, 256], DT8)
        wv_t = pp.tile([128, 8, 256], DT)
        wv_t8 = pp.tile([128, 8, 256], DT8)
        wo_t = pp.tile([128, 2, D], DT8)
        wo_t16 = pp.tile([128, 2, D], DT)

        # ---- stream tiles: [128, 4, 512] halves, triple buffered over sb so
        # the sb+2 prefetch DMA can start before sb's reads finish ----
        qsrc = qt.rearrange("(c p) s -> p c s", p=128)
        ksrc = kt.rearrange("(c p) s -> p c s", p=128)
        vsrc16 = vt16.rearrange("(c p) s -> p c s", p=128)
        vsrc8 = vt8.rearrange("(c p) s -> p c s", p=128)
        sstreams = {}

        SDT = {"q": DT8, "k": DT8, "v": DT}

        def v_stream_tile(sb, half):
            if sb == 0:
                tl = sp.tile([128, 4, 512], DT, tag=f"v16s{half}",
                             name=f"v16s{half}", bufs=1)
                src = vsrc16[:, ds(half * 4, 4), :].bitcast(DT)
            else:
                tl = sp.tile([128, 4, 512], DT8, tag=f"vs{half}",
                             name=f"vs{half}_{sb}", bufs=3)
                src = vsrc8[:, ds(half * 4, 4), ts(sb - 1, 512)]
            return tl, src

        def emit_stream_dma(sb):
            t = {}
            for name, src in (("q", qsrc), ("k", ksrc)):
                for half in range(2):
                    tl = sp.tile([128, 4, 512], SDT[name], tag=f"{name}s{half}",
                                 name=f"{name}s{half}_{sb}", bufs=3)
                    (nc.sync if half == 0 else nc.gpsimd).dma_start(
                        out=tl, in_=src[:, ds(half * 4, 4), ts(sb, 512)].bitcast(SDT[name]))
                    t[(name, half)] = tl
            for half in range(2):
                tl, src = v_stream_tile(sb, half)
                (nc.sync if half == 0 else nc.gpsimd).dma_start(out=tl, in_=src)
                t[("v", half)] = tl
            sstreams[sb] = t

        def chunk(sb, name, c):
            return sstreams[sb][(name, c // 4)][:, c % 4, :]

        # ---- compute units ----
        def chunk2(sb, name, j):
            # adjacent chunk pair (2j, 2j+1) as a [128, 2, 512] AP for DoubleRow
            half, r = divmod(2 * j, 4)
            return sstreams[sb][(name, half)][:, ds(r, 2), :]

        def psq_unit(sb, hp):
            ps = mm_ps.tile([128, 512], f32, tag="mm", name="psq")
            for j in range(4):
                nc.tensor.matmul(ps, wq_t[:, ds(2 * j, 2), ts(hp, 128)],
                                 chunk2(sb, "q", j),
                                 start=(j == 0), stop=(j == 3), perf_mode=DROW)
            nc.vector.tensor_scalar_add(q2t[hp][:, ts(sb, 512)], ps, bq_t[:, ds(hp, 1)])

        def psk_unit(sb, hp):
            ps = mm_ps.tile([128, 512], f32, tag="mm", name="psk")
            for j in range(4):
                nc.tensor.matmul(ps, wk_t[:, ds(2 * j, 2), ts(hp, 128)],
                                 chunk2(sb, "k", j),
                                 start=(j == 0), stop=(j == 3), perf_mode=DROW)
            nc.vector.tensor_scalar_add(k2t[hp][:, ts(sb, 512)], ps, bk_t[:, ds(hp, 1)])

        def psv_unit(sb, st4):
            st = sb * 4 + st4
            ps = mm_ps.tile([128, 256], f32, tag="mm", name="psv")
            if sb == 0:
                for c in range(8):
                    nc.tensor.matmul(ps, chunk(sb, "v", c)[:, ts(st4, 128)],
                                     wv_t[:, c, :], start=(c == 0), stop=(c == 7))
            else:
                for j in range(4):
                    half, r = divmod(2 * j, 4)
                    vpair = sstreams[sb][("v", half)][:, ds(r, 2), ts(st4, 128)]
                    nc.tensor.matmul(ps, vpair, wv_t8[:, ds(2 * j, 2), :],
                                     start=(j == 0), stop=(j == 3), perf_mode=DROW)
            bv3 = bv_bc.rearrange("p (h x) -> p h x", h=HPC)
            nc.vector.tensor_add(
                vaug[:, st, :].rearrange("p (h x) -> p h x", h=HPC)[:, :, 0:DH],
                ps.rearrange("p (h x) -> p h x", h=HPC), bv3)
            if st < 4:
                nc.vector.tensor_add(
                    vaug16[:, st, :].rearrange("p (h x) -> p h x", h=HPC)[:, :, 0:DH],
                    ps.rearrange("p (h x) -> p h x", h=HPC), bv3)

        def po_unit(st, nb):
            po = mm_ps.tile([128, 512], f32, tag="mm", name="po")
            if st < 4:
                for k in range(2):
                    nc.tensor.matmul(po, ctxt16[:, k, ts(st, 128)],
                                     wo_t16[:, k, ts(nb, 512)],
                                     start=(k == 0), stop=(k == 1))
            else:
                nc.tensor.matmul(po, ctxt[:, :, ts(st, 128)], wo_t[:, :, ts(nb, 512)],
                                 start=True, stop=True, perf_mode=DROW)
            ot = op.tile([128, 512], mybir.dt.float16, tag="ot", name="ot")
            nc.vector.tensor_copy(ot, po)
            nc.sync.dma_start(out=out[ts(st, 128), ts(nb, 512)], in_=ot)

        filler = []

        def emit_filler(n):
            for _ in range(n):
                if filler:
                    filler.pop(0)()

        # ---- prologue ----
        # ~5us of dummy matmuls: spans the DMA wait, flips HAM to full clock
        wps = sc_ps.tile([128, 1024], f32, tag="sc", name="wps")
        for _ in range(12):
            nc.tensor.matmul(wps[:, 0:512], warm[:, 0:128], warm,
                             start=True, stop=True)
        # DMA issue order == global service order == first-needed-first:
        # wq, qh0, qh1, wk, kh0, kh1, wv, vh0, vh1, then sb1; wo on the
        # lightly-used scalar queue so it doesn't delay the stream rails
        def _stile(name, half, sb):
            return sp.tile([128, 4, 512], SDT[name], tag=f"{name}s{half}",
                           name=f"{name}s{half}_{sb}", bufs=3)

        srcs = {"q": qsrc, "k": ksrc}
        t0 = {(n, h): _stile(n, h, 0) for n in "qk" for h in range(2)}
        for h in range(2):
            t0[("v", h)] = v_stream_tile(0, h)[0]
        sstreams[0] = t0
        nc.sync.dma_start(out=wq_t, in_=wq.rearrange("(c p) m -> p c m", p=128).bitcast(DT8))
        for i, (n, h) in enumerate((("q", 0), ("q", 1), ("k", 0), ("k", 1),
                                    ("v", 0), ("v", 1))):
            eng = nc.gpsimd if i % 2 == 0 else nc.sync
            if n == "v":
                src = vsrc16[:, ds(h * 4, 4), :].bitcast(DT)
            else:
                src = srcs[n][:, ds(h * 4, 4), ts(0, 512)].bitcast(SDT[n])
            eng.dma_start(out=t0[(n, h)], in_=src)
            if (n, h) == ("q", 1):
                nc.gpsimd.dma_start(
                    out=wk_t, in_=wk.rearrange("(c p) m -> p c m", p=128).bitcast(DT8))
            if (n, h) == ("k", 1):
                nc.sync.dma_start(
                    out=wv_t, in_=wv.rearrange("(c p) m -> p c m", p=128).bitcast(DT))
                nc.sync.dma_start(
                    out=wv_t8, in_=wv8.rearrange("(c p) m -> p c m", p=128))
        emit_stream_dma(1)
        # one-time broadcast of the V bias for the fused psv drain-add
        bv_bc = pp.tile([128, HPC * DH], DT, name="bv_bc")
        nc.gpsimd.partition_broadcast(bv_bc, bv_t, channels=128)
        psq_unit(0, 0)
        psk_unit(0, 0)

        # ---- main interleaved loop ----
        DEPTH = 4  # attnV deferral depth (pairs)
        for sb in range(NSB):
            if sb < NSB - 2:
                emit_stream_dma(sb + 2)
            if sb == 1:
                # wo loads deferred here so they don't delay the early streams
                nc.gpsimd.dma_start(out=wo_t, in_=wo.rearrange("h p n -> p h n"))
                nc.gpsimd.dma_start(
                    out=wo_t16, in_=wo16.rearrange("h p n -> p h n").bitcast(DT))
            pos = [(st, nb) for st in range(4 * (sb - 1), 4 * sb)
                   for nb in range(2)] if sb >= 1 else []
            for hp in range(2):
                # refill filler queue for this hp
                if sb == 0 and hp == 0:
                    filler.extend([lambda t=t: psv_unit(0, t) for t in range(4)])
                    filler.extend([lambda: psq_unit(0, 1), lambda: psk_unit(0, 1)])
                    filler.extend(
                        [lambda: psq_unit(1, 0), lambda: psk_unit(1, 0),
                         lambda: psq_unit(1, 1), lambda: psk_unit(1, 1)])
                elif hp == 0:
                    filler.extend(
                        (lambda st=st, nb=nb: po_unit(st, nb)) for st, nb in pos[:4])
                    if sb < NSB - 1:
                        nsb = sb + 1
                        filler.extend(
                            [lambda s=nsb: psq_unit(s, 0), lambda s=nsb: psk_unit(s, 0),
                             lambda s=nsb: psq_unit(s, 1), lambda s=nsb: psk_unit(s, 1)])
                if hp == 1:
                    filler.extend(
                        (lambda st=st, nb=nb: po_unit(st, nb)) for st, nb in pos[4:])
                    if sb < NSB - 1:
                        nsb = sb + 1
                        filler.extend(
                            lambda s=nsb, t=t: psv_unit(s, t) for t in range(4))

                jts = list(range(4 * sb + 4))
                nj = len(jts)
                cps = [ctx_ps.tile([HB, 512], f32, tag=f"ctx{a}", name=f"cps{a}")
                       for a in range(2)]
                pend = []
                cur = None

                def lo_of(j):
                    return max(0, (j - 4 * sb) * 128)

                def emit_attnv(pj0, pat, last):
                    # fp8 DoubleRow over the jt pair (pj0, pj0+1): 256-deep
                    # key contraction in one pass
                    lo = lo_of(pj0)
                    for a in range(2):
                        h = 2 * hp + a
                        nc.tensor.matmul(
                            cps[a][:, ds(lo, 512 - lo)],
                            vaug[:, ds(pj0, 2), ds(h * HB, HB)],
                            pat[:, :, ds(a * 512 + lo, 512 - lo)],
                            start=(pj0 == 0), stop=last, perf_mode=DROW)

                def emit_attnv16(pjt, pat, last):
                    # f16 single-jt path for the first query block
                    lo = lo_of(pjt)
                    for a in range(2):
                        h = 2 * hp + a
                        nc.tensor.matmul(
                            cps[a][0:DH + 1, ds(lo, 512 - lo)],
                            vaug16[:, pjt, ds(h * 65, DH + 1)],
                            pat[:, ds(a * 512 + lo, 512 - lo)],
                            start=(pjt == 0), stop=last)

                emit_av = emit_attnv16 if sb == 0 else emit_attnv

                for ji, jt in enumerate(jts):
                    if ji < 2:
                        emit_filler(1)
                    else:
                        left = nj - ji
                        n = max(1, (len(filler) + left - 1) // left) if filler else 0
                        emit_filler(n)
                    straddle = jt >= 4 * sb
                    lo = lo_of(jt)
                    sc = sc_ps.tile([128, 1024], f32, tag="sc")
                    for a in range(2):
                        nc.tensor.matmul(
                            sc[:, ds(a * 512 + lo, 512 - lo)],
                            k2t[hp][ds(a * 64, 64), ts(jt, 128)],
                            q2t[hp][ds(a * 64, 64), ds(sb * 512 + lo, 512 - lo)],
                            start=True, stop=True,
                            tile_position=(a * 64, 0))
                    if sb == 0:
                        at = ap.tile([128, 1024], DT, tag="at0", name="at0",
                                     bufs=4)
                        if lo == 0:
                            nc.scalar.activation(at, sc, EXP, scale=0.125)
                        else:
                            nc.scalar.activation(
                                at.rearrange("p (a n) -> p a n", a=2)
                                [:, :, ds(lo, 512 - lo)],
                                sc.rearrange("p (a n) -> p a n", a=2)
                                [:, :, ds(lo, 512 - lo)],
                                EXP, scale=0.125)
                        # post-exp causal zeroing of the diagonal block on the
                        # pool engine (SBUF-only, so gpsimd can do it)
                        atv = at.rearrange("p (a n) -> p a n", a=2)[:, :, ds(lo, 128)]
                        nc.gpsimd.tensor_mul(atv, atv, mtri_t)
                        pend.append((jt, at))
                        if len(pend) > DEPTH:
                            j0, a0 = pend.pop(0)
                            emit_av(j0, a0, False)
                        continue
                    m = ji % 2
                    if m == 0:
                        cur = ap.tile([128, 2, 1024], DT8, tag="atf",
                                      name="atf", bufs=6)
                        if straddle:
                            # zero the pair partner's causally-dead columns on
                            # the (idle) pool engine so they contribute nothing
                            # to the DoubleRow attnV contraction
                            z0 = lo  # partner's dead region is [lo, lo+128)
                            for a in range(2):
                                nc.gpsimd.memset(
                                    cur[:, 1, ds(a * 512 + z0, 128)], 0.0)
                    # fastexp: DVE int8 Schraudolph whose bits ARE the fp8e4m3
                    # result. Used where the ACT backlog would stall the tensor
                    # engine: each block's first pair and the terminal drain.
                    fastexp = (jt <= 1 or
                               (sb == NSB - 1 and hp == 1 and 2 <= jt <= 7))
                    if fastexp:
                        # int8 convert truncates toward zero (no rounding), so
                        # bias the Schraudolph constant by +0.5
                        nc.vector.tensor_scalar(
                            cur[:, m, :].bitcast(mybir.dt.int8), sc,
                            0.125 * 8.0 / 0.6931471805599453, 56.15,
                            op0=mybir.AluOpType.mult, op1=mybir.AluOpType.add)
                    elif lo == 0:
                        nc.scalar.activation(cur[:, m, :], sc, EXP, scale=0.125)
                    else:
                        nc.scalar.activation(
                            cur[:, m, :].rearrange("p (a n) -> p a n", a=2)
                            [:, :, ds(lo, 512 - lo)],
                            sc.rearrange("p (a n) -> p a n", a=2)
                            [:, :, ds(lo, 512 - lo)],
                            EXP, scale=0.125)
                    if straddle:
                        atv = (cur[:, m, :].rearrange("p (a n) -> p a n", a=2)
                               [:, :, ds(lo, 128)])
                        nc.gpsimd.tensor_mul(atv, atv, mtri_t)
                    if m == 1:
                        pend.append((jt - 1, cur))
                        if len(pend) > DEPTH:
                            j0, a0 = pend.pop(0)
                            emit_av(j0, a0, False)
                while pend:
                    j0, a0 = pend.pop(0)
                    emit_av(j0, a0, not pend)
                # softmax normalization: 1/denom broadcast-multiplied into ctxt
                for a in range(2):
                    # copy denom to SBUF first: reciprocal_approx_fast is a
                    # bitwise trick and must not read the PSUM port directly
                    dn0 = smp.tile([1, 512], f32, tag="dn0", name="dn0")
                    nc.vector.tensor_copy(dn0, cps[a][ds(DH, 1), :])
                    rd = smp.tile([1, 512], f32, tag="rd", name="rd")
                    nc.vector.reciprocal_approx_fast(rd, dn0)
                    bc = smp.tile([DH, 512], f32, tag="bc", name="bc")
                    nc.gpsimd.partition_broadcast(bc, rd, channels=DH)
                    cdst = (ctxt16[ds(a * DH, DH), hp, :] if sb == 0 else
                            ctxt[ds(a * DH, DH), hp, ts(sb, 512)])
                    nc.vector.tensor_mul(cdst, cps[a][0:DH, :], bc)

        # drain remaining fillers + final output projection
        emit_filler(len(filler))
        for st in range(4 * (NSB - 1), 4 * NSB):
            for nb in range(2):
                po_unit(st, nb)

        cx.close()

    nc.finalize()
    _built[key] = nc
    return nc


def _is_causal(masked: np.ndarray) -> bool:
    c = ~np.tril(np.ones((S, S), dtype=bool))
    return all(np.array_equal(masked[b], c) for b in range(masked.shape[0]))


def kernel(Q, K, V, masked, WQ_w, WQ_b, WK_w, WK_b, WV_w, WV_b, Wo_w, Wo_b):
    global LAST_RESULT
    from concourse.bass_utils import run_bass_kernel_spmd

    Q = np.asarray(Q, dtype=np.float32)
    K = np.asarray(K, dtype=np.float32)
    V = np.asarray(V, dtype=np.float32)
    masked = np.asarray(masked)
    causal = _is_causal(masked)
    if not causal:
        return _kernel_legacy(Q, K, V, masked, WQ_w, WQ_b, WK_w, WK_b,
                              WV_w, WV_b, Wo_w, Wo_b)
    nc = _build_v2(MMDT)
    if MMDT == "f16":
        npdt = np.float16
    elif MMDT == "bf16":
        import ml_dtypes
        npdt = ml_dtypes.bfloat16
    else:
        npdt = np.float32

    import ml_dtypes
    f8 = ml_dtypes.float8_e4m3fn
    qT = [np.ascontiguousarray(Q[b].T.astype(f8)) for b in range(B)]
    kT = [np.ascontiguousarray(K[b].T.astype(f8)) for b in range(B)]
    vT16 = [np.ascontiguousarray(V[b, :512].T.astype(npdt)) for b in range(B)]
    vT8 = [np.ascontiguousarray(V[b, 512:].T.astype(f8)) for b in range(B)]

    j = np.arange(128)[:, None]
    c = np.arange(128)[None, :]
    mtri_1 = (j <= c).astype(f8)  # 0/1 keep-mask: key j visible to query c
    mtri_full = np.ascontiguousarray(
        np.broadcast_to(mtri_1[:, None, :], (128, 2, 128)))

    in_maps = []
    for cc in range(NCORES):
        b = cc // CORES_PER_BATCH
        h0 = (cc % CORES_PER_BATCH) * HPC
        sel = slice(h0 * DH, (h0 + HPC) * DH)
        wo_pad = np.asarray(Wo_w).T[sel].reshape(2, 128, D).astype(np.float32)
        m = {
            "qt": qT[b], "kt": kT[b], "vt16": vT16[b], "vt8": vT8[b],
            "wq": np.ascontiguousarray(np.asarray(WQ_w)[sel].T.astype(f8)),
            "wk": np.ascontiguousarray(np.asarray(WK_w)[sel].T.astype(f8)),
            "wv": np.ascontiguousarray(np.asarray(WV_w)[sel].T.astype(npdt)),
            "wv8": np.ascontiguousarray(np.asarray(WV_w)[sel].T.astype(f8)),
            "wo": wo_pad.astype(f8),
            "wo16": wo_pad.astype(npdt),
            "bq": np.ascontiguousarray(np.asarray(WQ_b)[sel].reshape(2, 128).T.astype(np.float32)),
            "bk": np.ascontiguousarray(np.asarray(WK_b)[sel].reshape(2, 128).T.astype(np.float32)),
            "bv": np.ascontiguousarray(np.asarray(WV_b)[sel].reshape(1, HPC * DH).astype(npdt)),
            "mtri": mtri_full,
        }
        m = {k: np.ascontiguousarray(v) for k, v in m.items()}
        in_maps.append(m)

    res = run_bass_kernel_spmd(nc, in_maps, core_ids=list(range(NCORES)), trace=TRACE)
    LAST_RESULT = res

    acc = np.zeros((B, S, D), np.float64)
    for cc in range(NCORES):
        acc[cc // CORES_PER_BATCH] += res.results[cc]["out"].astype(np.float64)
    acc += np.asarray(Wo_b, dtype=np.float64)[None, None, :]
    return acc.astype(np.float32)


# ---------------------------------------------------------------------------
# legacy non-causal fallback (general mask multiply path)
# ---------------------------------------------------------------------------

def _build_legacy(mmdt: str):
    key = ("legacy", mmdt)
    if key in _built:
        return _built[key]
    import concourse.mybir as mybir
    import concourse.tile as tile
    from concourse import bacc
    from concourse.bass import ts, ds

    f32 = mybir.dt.float32
    DT = {"f32r": mybir.dt.float32r, "f16": mybir.dt.float16,
          "bf16": mybir.dt.bfloat16}[mmdt]
    DTNP = {"f32r": f32, "f16": mybir.dt.float16, "bf16": mybir.dt.bfloat16}[mmdt]
    EXP = mybir.ActivationFunctionType.Exp

    nc = bacc.Bacc("TRN2")
    qt = nc.dram_tensor("qt", [D, S], DTNP, kind="ExternalInput")
    kt = nc.dram_tensor("kt", [D, S], DTNP, kind="ExternalInput")
    vt = nc.dram_tensor("vt", [D, S], DTNP, kind="ExternalInput")
    wq = nc.dram_tensor("wq", [D, HPC * DH], DTNP, kind="ExternalInput")
    wk = nc.dram_tensor("wk", [D, HPC * DH], DTNP, kind="ExternalInput")
    wv = nc.dram_tensor("wv", [D, HPC * DH], DTNP, kind="ExternalInput")
    wo = nc.dram_tensor("wo", [2, 128, D], DTNP, kind="ExternalInput")
    bq = nc.dram_tensor("bq", [128, 2], f32, kind="ExternalInput")
    bk = nc.dram_tensor("bk", [128, 2], f32, kind="ExternalInput")
    bv = nc.dram_tensor("bv", [1, HPC * DH], DTNP, kind="ExternalInput")
    mt = nc.dram_tensor("mt", [S, S], DTNP, kind="ExternalInput")
    out = nc.dram_tensor("out", [S, D], f32, kind="ExternalOutput")

    NSB = S // 512
    NST = S // 128

    import contextlib
    with tile.TileContext(nc) as tc, contextlib.ExitStack() as ctx_pools:
        with (
            tc.tile_pool(name="persist", bufs=1) as pp,
            tc.tile_pool(name="sc_ps", bufs=2, space="PSUM") as sc_ps,
            tc.tile_pool(name="ctx_ps", bufs=1, space="PSUM") as ctx_ps,
            tc.tile_pool(name="mm_ps", bufs=2, space="PSUM") as mm_ps,
        ):
            bq_t = pp.tile([128, 2], f32)
            nc.gpsimd.dma_start(out=bq_t, in_=bq[:, :])
            bk_t = pp.tile([128, 2], f32)
            nc.gpsimd.dma_start(out=bk_t, in_=bk[:, :])
            bv_t = pp.tile([1, HPC * DH], DT)
            nc.gpsimd.dma_start(out=bv_t, in_=bv[:, :].bitcast(DT))

            ones_c = pp.tile([1, 128], DT)
            nc.vector.memset(ones_c.bitcast(DTNP), 1.0)

            ap = ctx_pools.enter_context(tc.tile_pool(name="attn", bufs=3))
            smp = ctx_pools.enter_context(tc.tile_pool(name="small", bufs=3))
            mlp = ctx_pools.enter_context(tc.tile_pool(name="mload", bufs=3))
            owp = ctx_pools.enter_context(tc.tile_pool(name="outw", bufs=1))
            op = ctx_pools.enter_context(tc.tile_pool(name="outp", bufs=4))

            q2t = [pp.tile([128, S], DT, tag=f"q2t{i}", name=f"q2t{i}") for i in range(2)]
            k2t = [pp.tile([128, S], DT, tag=f"k2t{i}", name=f"k2t{i}") for i in range(2)]
            vaug = pp.tile([128, NST, HPC * (DH + 1)], DT)
            for h in range(HPC):
                nc.vector.memset(vaug[:, :, ds(h * 65 + 64, 1)].bitcast(DTNP), 1.0)
            ctxt = [pp.tile([128, S], DT, tag=f"ctxt{i}", name=f"ctxt{i}") for i in range(2)]

            with (
                tc.tile_pool(name="wproj", bufs=1) as wp,
                tc.tile_pool(name="stream", bufs=2) as sp,
            ):
                wq_t = wp.tile([128, 8, 256], DT)
                nc.gpsimd.dma_start(out=wq_t, in_=wq.rearrange("(c p) m -> p c m", p=128).bitcast(DT))
                wk_t = wp.tile([128, 8, 256], DT)
                nc.gpsimd.dma_start(out=wk_t, in_=wk.rearrange("(c p) m -> p c m", p=128).bitcast(DT))
                wv_t = wp.tile([128, 8, 256], DT)
                nc.gpsimd.dma_start(out=wv_t, in_=wv.rearrange("(c p) m -> p c m", p=128).bitcast(DT))

                for sb in range(NSB):
                    qs = [sp.tile([128, 4, 512], DT, tag=f"qs{i}", name=f"qs{i}", bufs=1) for i in range(2)]
                    ks = [sp.tile([128, 4, 512], DT, tag=f"ks{i}", name=f"ks{i}", bufs=1) for i in range(2)]
                    vs = [sp.tile([128, 4, 512], DT, tag=f"vs{i}", name=f"vs{i}", bufs=1) for i in range(2)]
                    for half in range(2):
                        for name, t, dr, eng in (("q", qs, qt, nc.sync),
                                                 ("k", ks, kt, nc.sync),
                                                 ("v", vs, vt, nc.gpsimd)):
                            src = dr.rearrange("(c p) s -> p c s", p=128)
                            eng.dma_start(
                                out=t[half],
                                in_=src[:, ds(half * 4, 4), ts(sb, 512)].bitcast(DT),
                            )
                    for hp in range(2):
                        psq = mm_ps.tile([128, 512], f32, tag="mm")
                        for c in range(8):
                            nc.tensor.matmul(
                                psq, wq_t[:, c, ts(hp, 128)], qs[c // 4][:, c % 4, :],
                                start=(c == 0), stop=(c == 7),
                            )
                        nc.vector.tensor_scalar_add(
                            q2t[hp][:, ts(sb, 512)], psq, bq_t[:, ds(hp, 1)])
                        psk = mm_ps.tile([128, 512], f32, tag="mm")
                        for c in range(8):
                            nc.tensor.matmul(
                                psk, wk_t[:, c, ts(hp, 128)], ks[c // 4][:, c % 4, :],
                                start=(c == 0), stop=(c == 7),
                            )
                        nc.vector.tensor_scalar_add(
                            k2t[hp][:, ts(sb, 512)], psk, bk_t[:, ds(hp, 1)])
                    for st4 in range(4):
                        st = sb * 4 + st4
                        psv = mm_ps.tile([128, 256], f32, tag="mm")
                        for c in range(8):
                            nc.tensor.matmul(
                                psv, vs[c // 4][:, c % 4, ts(st4, 128)], wv_t[:, c, :],
                                start=(c == 0), stop=False,
                            )
                        nc.tensor.matmul(psv, ones_c, bv_t, start=False, stop=True)
                        nc.vector.tensor_copy(
                            vaug[:, st, :].rearrange("p (h x) -> p h x", h=HPC)[:, :, 0:DH],
                            psv.rearrange("p (h x) -> p h x", h=HPC),
                        )

            wo_t = owp.tile([128, 2, D], DT)
            nc.gpsimd.dma_start(out=wo_t, in_=wo.rearrange("h p n -> p h n").bitcast(DT))
            outq = []

            def emit_out_unit():
                if not outq:
                    return
                st, nb = outq.pop(0)
                po = mm_ps.tile([128, 512], f32, tag="mm", name="po")
                for k in range(2):
                    nc.tensor.matmul(
                        po, ctxt[k][:, ts(st, 128)],
                        wo_t[:, k, ts(nb, 512)],
                        start=(k == 0), stop=(k == 1),
                    )
                ot = op.tile([128, 512], f32, tag="ot", name="ot")
                nc.any.tensor_copy(ot, po)
                (nc.gpsimd if (st + nb) % 2 else nc.sync).dma_start(out=out[ts(st, 128), ts(nb, 512)], in_=ot)

            for sb in range(NSB):
                for hp in range(2):
                    jts = list(range(NST))
                    cps = [ctx_ps.tile([DH + 1, 512], f32, tag=f"ctx{a}", name=f"cps{a}")
                           for a in range(2)]
                    pending = None

                    def emit_attnv(pjt, pat, last):
                        for a in range(2):
                            h = 2 * hp + a
                            nc.tensor.matmul(
                                cps[a],
                                vaug[:, pjt, ds(h * 65, DH + 1)],
                                pat[:, ds(a * 512, 512)],
                                start=(pjt == jts[0]), stop=last,
                            )

                    for jt in jts:
                        emit_out_unit()
                        sc = sc_ps.tile([128, 1024], f32, tag="sc")
                        mt_t = mlp.tile([128, 512], DT, tag="mt")
                        nc.sync.dma_start(
                            out=mt_t,
                            in_=mt[ts(jt, 128), ts(sb, 512)].bitcast(DT))
                        for a in range(2):
                            nc.tensor.matmul(
                                sc[:, ds(a * 512, 512)],
                                k2t[hp][ds(a * 64, 64), ts(jt, 128)],
                                q2t[hp][ds(a * 64, 64), ds(sb * 512, 512)],
                                start=True, stop=True,
                                tile_position=(a * 64, 0),
                            )
                        at = ap.tile([128, 1024], DT, tag="at")
                        nc.scalar.activation(at, sc, EXP, scale=0.125)
                        for a in range(2):
                            nc.vector.tensor_mul(
                                at[:, ts(a, 512)], at[:, ts(a, 512)], mt_t)
                        if pending is not None:
                            emit_attnv(pending[0], pending[1], False)
                        pending = (jt, at)
                    emit_attnv(pending[0], pending[1], True)
                    for a in range(2):
                        dn0 = smp.tile([1, 512], f32, tag="dn0", name="dn0")
                        nc.vector.tensor_copy(dn0, cps[a][ds(DH, 1), :])
                        rd = smp.tile([1, 512], f32, tag="rd", name="rd")
                        nc.vector.reciprocal_approx_fast(rd, dn0)
                        bc = smp.tile([DH, 512], f32, tag="bc", name="bc")
                        nc.gpsimd.partition_broadcast(bc, rd, channels=DH)
                        nc.vector.tensor_mul(
                            ctxt[hp][ds(a * DH, DH), ts(sb, 512)],
                            cps[a][0:DH, :], bc)
                outq.extend((st, nb) for st in range(4 * sb, 4 * sb + 4)
                            for nb in range(2))
            while outq:
                emit_out_unit()

            ctx_pools.close()

    nc.finalize()
    _built[key] = nc
    return nc


def _kernel_legacy(Q, K, V, masked, WQ_w, WQ_b, WK_w, WK_b, WV_w, WV_b, Wo_w, Wo_b):
    global LAST_RESULT
    from concourse.bass_utils import run_bass_kernel_spmd

    nc = _build_legacy(MMDT)
    if MMDT == "f16":
        npdt = np.float16
    elif MMDT == "bf16":
        import ml_dtypes
        npdt = ml_dtypes.bfloat16
    else:
        npdt = np.float32

    qT = [np.ascontiguousarray(Q[b].T.astype(npdt)) for b in range(B)]
    kT = [np.ascontiguousarray(K[b].T.astype(npdt)) for b in range(B)]
    vT = [np.ascontiguousarray(V[b].T.astype(npdt)) for b in range(B)]
    mtb = [np.ascontiguousarray(
        np.where(masked[b].T, np.float32(0.0), np.float32(1.0)).astype(npdt))
        for b in range(B)]

    in_maps = []
    for c in range(NCORES):
        b = c // CORES_PER_BATCH
        h0 = (c % CORES_PER_BATCH) * HPC
        sel = slice(h0 * DH, (h0 + HPC) * DH)
        wo_pad = np.asarray(Wo_w).T[sel].reshape(2, 128, D).astype(np.float32)
        m = {
            "qt": qT[b], "kt": kT[b], "vt": vT[b],
            "wq": np.ascontiguousarray(np.asarray(WQ_w)[sel].T.astype(npdt)),
            "wk": np.ascontiguousarray(np.asarray(WK_w)[sel].T.astype(npdt)),
            "wv": np.ascontiguousarray(np.asarray(WV_w)[sel].T.astype(npdt)),
            "wo": wo_pad.astype(npdt),
            "bq": np.ascontiguousarray(np.asarray(WQ_b)[sel].reshape(2, 128).T.astype(np.float32)),
            "bk": np.ascontiguousarray(np.asarray(WK_b)[sel].reshape(2, 128).T.astype(np.float32)),
            "bv": np.ascontiguousarray(np.asarray(WV_b)[sel].reshape(1, HPC * DH).astype(npdt)),
            "mt": mtb[b],
        }
        m = {k: np.ascontiguousarray(v) for k, v in m.items()}
        in_maps.append(m)

    res = run_bass_kernel_spmd(nc, in_maps, core_ids=list(range(NCORES)), trace=TRACE)
    LAST_RESULT = res

    acc = np.zeros((B, S, D), np.float64)
    for c in range(NCORES):
        acc[c // CORES_PER_BATCH] += res.results[c]["out"].astype(np.float64)
    acc += np.asarray(Wo_b, dtype=np.float64)[None, None, :]
    return acc.astype(np.float32)



# revision 66
# speedup vs baseline: 1.2560x; 1.2560x over previous
"""Multi-head attention (B=2, S=2048, D=1024, H=16) on 8 TRN2 NeuronCores.

Sharding: batch (2) x head-groups (4 heads/core). Each core computes its
batch's QKV projections for its 4 heads, causal attention, and a partial
output projection over its head slice; the host sums the 4 partials per
batch and adds the output bias.

Layout strategy: everything runs in "transposed" orientation so no on-chip
transposes are needed:
  q2^T[dm, s] = Wq[dm,:] @ Q^T       (host supplies Q^T and Wq^T)
  scores^T[j, si] = k^T.T @ q^T      (d_h contraction, 2 heads row-tiled)
  attn^T = exp(scores^T/8 + mask)    (mask added pre-exp via -60000*tri matmul)
  ctx^T+denom = [v | 1].T @ attn^T   (ones column gives softmax denominator)
  out[s, n] = ctxn^T.T @ Wo^T        (K=128 chunks x2)
v2: single interleaved pipeline -- QKV projection and output projection
units are emitted as fillers inside the attention jt loop so the tensor
queue never drains (keeps HAM at full clock); mask applied pre-exp on the
tensor engine; straddle exp in one strided-AP instruction; f16 output.
"""

import numpy as np

B, S, D, H, DH = 2, 2048, 1024, 16, 64
NCORES = 8
CORES_PER_BATCH = 4
HPC = H // CORES_PER_BATCH  # heads per core = 4
NEG = -60000.0  # exp((x+NEG)/8) == 0 exactly in fp32; fits in fp16
MMDT = "f16"

TRACE = False  # test.py sets True to collect an NTFF profile
LAST_RESULT = None  # BassKernelResults from the last run (for test.py)

_built = {}


def _build_v2(mmdt: str):
    key = ("v2", mmdt)
    if key in _built:
        return _built[key]
    import concourse.mybir as mybir
    import concourse.tile as tile
    from concourse import bacc
    from concourse.bass import ts, ds

    f32 = mybir.dt.float32
    DT = {"f32r": mybir.dt.float32r, "f16": mybir.dt.float16,
          "bf16": mybir.dt.bfloat16}[mmdt]
    DTNP = {"f32r": f32, "f16": mybir.dt.float16, "bf16": mybir.dt.bfloat16}[mmdt]
    EXP = mybir.ActivationFunctionType.Exp
    DT8 = mybir.dt.float8e4
    DROW = mybir.MatmulPerfMode.DoubleRow

    nc = bacc.Bacc("TRN2")
    qt = nc.dram_tensor("qt", [D, S], DT8, kind="ExternalInput")
    kt = nc.dram_tensor("kt", [D, S], DT8, kind="ExternalInput")
    # V split by key range: keys < 512 stay f16 (feed the exact early-query
    # path); keys >= 512 are fp8 (only ever read by spread-attention queries)
    vt16 = nc.dram_tensor("vt16", [D, 512], DTNP, kind="ExternalInput")
    vt8 = nc.dram_tensor("vt8", [D, S - 512], DT8, kind="ExternalInput")
    wq = nc.dram_tensor("wq", [D, HPC * DH], DT8, kind="ExternalInput")
    wk = nc.dram_tensor("wk", [D, HPC * DH], DT8, kind="ExternalInput")
    wv = nc.dram_tensor("wv", [D, HPC * DH], DTNP, kind="ExternalInput")
    wv8 = nc.dram_tensor("wv8", [D, HPC * DH], DT8, kind="ExternalInput")
    wo = nc.dram_tensor("wo", [2, 128, D], DT8, kind="ExternalInput")
    wo16 = nc.dram_tensor("wo16", [2, 128, D], DTNP, kind="ExternalInput")
    bq = nc.dram_tensor("bq", [128, 2], f32, kind="ExternalInput")
    bk = nc.dram_tensor("bk", [128, 2], f32, kind="ExternalInput")
    bv = nc.dram_tensor("bv", [1, HPC * DH], DTNP, kind="ExternalInput")
    mtri = nc.dram_tensor("mtri", [128, 2, 128], DTNP, kind="ExternalInput")
    out = nc.dram_tensor("out", [S, D], mybir.dt.float16, kind="ExternalOutput")

    NSB = S // 512   # 4 si-blocks of 512
    NST = S // 128   # 16 s-tiles / j-tiles of 128

    import contextlib
    with tile.TileContext(nc) as tc, contextlib.ExitStack() as cx:
        pp = cx.enter_context(tc.tile_pool(name="persist", bufs=1))
        sc_ps = cx.enter_context(tc.tile_pool(name="sc_ps", bufs=2, space="PSUM"))
        ctx_ps = cx.enter_context(tc.tile_pool(name="ctx_ps", bufs=1, space="PSUM"))
        mm_ps = cx.enter_context(tc.tile_pool(name="mm_ps", bufs=2, space="PSUM"))
        sp = cx.enter_context(tc.tile_pool(name="stream", bufs=1))
        ap = cx.enter_context(tc.tile_pool(name="attn", bufs=4))
        smp = cx.enter_context(tc.tile_pool(name="small", bufs=3))
        op = cx.enter_context(tc.tile_pool(name="outp", bufs=4))

        # ---- persistent tiles + constant DMAs (small, on scalar queue) ----
        bq_t = pp.tile([128, 2], f32)
        nc.scalar.dma_start(out=bq_t, in_=bq[:, :])
        bk_t = pp.tile([128, 2], f32)
        nc.scalar.dma_start(out=bk_t, in_=bk[:, :])
        bv_t = pp.tile([1, HPC * DH], DT)
        nc.scalar.dma_start(out=bv_t, in_=bv[:, :].bitcast(DT))
        mtri_t = pp.tile([128, 2, 128], DT)
        nc.scalar.dma_start(out=mtri_t, in_=mtri[:, :, :].bitcast(DT))

        ones_c = pp.tile([1, 128], DT)
        nc.vector.memset(ones_c.bitcast(DTNP), 1.0)
        warm = pp.tile([128, 512], DT, name="warm")
        nc.vector.memset(warm.bitcast(DTNP), 1.0)

        q2t = [pp.tile([128, S], DT, tag=f"q2t{i}", name=f"q2t{i}") for i in range(2)]
        k2t = [pp.tile([128, S], DT, tag=f"k2t{i}", name=f"k2t{i}") for i in range(2)]
        # fp8 vaug in 96-wide head blocks (v 0:64 | ones 64 | zero pad 65:96):
        # dual-fp8 ldweights requires a 32-multiple stationary free size
        HB = 96
        vaug = pp.tile([128, NST, HPC * HB], DT8)
        for h in range(HPC):
            nc.vector.memset(vaug[:, :, ds(h * HB + 64, 1)], 1.0)
            nc.vector.memset(vaug[:, :, ds(h * HB + 65, HB - 65)], 0.0)
        # f16 copies for the first query block (sb=0): near-one-hot attention
        # there makes ctx ~ a raw V row, so fp8 V/ctx/Wo would be ~4% off
        vaug16 = pp.tile([128, 4, HPC * (DH + 1)], DT, name="vaug16")
        for h in range(HPC):
            nc.vector.memset(vaug16[:, :, ds(h * 65 + 64, 1)].bitcast(DTNP), 1.0)
        ctxt = pp.tile([128, 2, S], DT8, name="ctxt")
        ctxt16 = pp.tile([128, 2, 512], DT, name="ctxt16")

        wq_t = pp.tile([128, 8, 256], DT8)
        wk_t = pp.tile([128, 8, 256], DT8)
        wv_t = pp.tile([128, 8, 256], DT)
        wv_t8 = pp.tile([128, 8, 256], DT8)
        wo_t = pp.tile([128, 2, D], DT8)
        wo_t16 = pp.tile([128, 2, D], DT)

        # ---- stream tiles: [128, 4, 512] halves, triple buffered over sb so
        # the sb+2 prefetch DMA can start before sb's reads finish ----
        qsrc = qt.rearrange("(c p) s -> p c s", p=128)
        ksrc = kt.rearrange("(c p) s -> p c s", p=128)
        vsrc16 = vt16.rearrange("(c p) s -> p c s", p=128)
        vsrc8 = vt8.rearrange("(c p) s -> p c s", p=128)
        sstreams = {}

        SDT = {"q": DT8, "k": DT8, "v": DT}

        def v_stream_tile(sb, half):
            if sb == 0:
                tl = sp.tile([128, 4, 512], DT, tag=f"v16s{half}",
                             name=f"v16s{half}", bufs=1)
                src = vsrc16[:, ds(half * 4, 4), :].bitcast(DT)
            else:
                tl = sp.tile([128, 4, 512], DT8, tag=f"vs{half}",
                             name=f"vs{half}_{sb}", bufs=3)
                src = vsrc8[:, ds(half * 4, 4), ts(sb - 1, 512)]
            return tl, src

        def emit_stream_dma(sb):
            t = {}
            for name, src in (("q", qsrc), ("k", ksrc)):
                for half in range(2):
                    tl = sp.tile([128, 4, 512], SDT[name], tag=f"{name}s{half}",
                                 name=f"{name}s{half}_{sb}", bufs=3)
                    (nc.sync if half == 0 else nc.gpsimd).dma_start(
                        out=tl, in_=src[:, ds(half * 4, 4), ts(sb, 512)].bitcast(SDT[name]))
                    t[(name, half)] = tl
            for half in range(2):
                tl, src = v_stream_tile(sb, half)
                (nc.sync if half == 0 else nc.gpsimd).dma_start(out=tl, in_=src)
                t[("v", half)] = tl
            sstreams[sb] = t

        def chunk(sb, name, c):
            return sstreams[sb][(name, c // 4)][:, c % 4, :]

        # ---- compute units ----
        def chunk2(sb, name, j):
            # adjacent chunk pair (2j, 2j+1) as a [128, 2, 512] AP for DoubleRow
            half, r = divmod(2 * j, 4)
            return sstreams[sb][(name, half)][:, ds(r, 2), :]

        def psq_unit(sb, hp):
            ps = mm_ps.tile([128, 512], f32, tag="mm", name="psq")
            for j in range(4):
                nc.tensor.matmul(ps, wq_t[:, ds(2 * j, 2), ts(hp, 128)],
                                 chunk2(sb, "q", j),
                                 start=(j == 0), stop=(j == 3), perf_mode=DROW)
            nc.vector.tensor_scalar_add(q2t[hp][:, ts(sb, 512)], ps, bq_t[:, ds(hp, 1)])

        def psk_unit(sb, hp):
            ps = mm_ps.tile([128, 512], f32, tag="mm", name="psk")
            for j in range(4):
                nc.tensor.matmul(ps, wk_t[:, ds(2 * j, 2), ts(hp, 128)],
                                 chunk2(sb, "k", j),
                                 start=(j == 0), stop=(j == 3), perf_mode=DROW)
            nc.vector.tensor_scalar_add(k2t[hp][:, ts(sb, 512)], ps, bk_t[:, ds(hp, 1)])

        def psv_unit(sb, st4):
            st = sb * 4 + st4
            ps = mm_ps.tile([128, 256], f32, tag="mm", name="psv")
            if sb == 0:
                for c in range(8):
                    nc.tensor.matmul(ps, chunk(sb, "v", c)[:, ts(st4, 128)],
                                     wv_t[:, c, :], start=(c == 0), stop=(c == 7))
            else:
                for j in range(4):
                    half, r = divmod(2 * j, 4)
                    vpair = sstreams[sb][("v", half)][:, ds(r, 2), ts(st4, 128)]
                    nc.tensor.matmul(ps, vpair, wv_t8[:, ds(2 * j, 2), :],
                                     start=(j == 0), stop=(j == 3), perf_mode=DROW)
            bv3 = bv_bc.rearrange("p (h x) -> p h x", h=HPC)
            nc.vector.tensor_add(
                vaug[:, st, :].rearrange("p (h x) -> p h x", h=HPC)[:, :, 0:DH],
                ps.rearrange("p (h x) -> p h x", h=HPC), bv3)
            if st < 4:
                nc.vector.tensor_add(
                    vaug16[:, st, :].rearrange("p (h x) -> p h x", h=HPC)[:, :, 0:DH],
                    ps.rearrange("p (h x) -> p h x", h=HPC), bv3)

        def po_unit(st, nb):
            po = mm_ps.tile([128, 512], f32, tag="mm", name="po")
            if st < 4:
                for k in range(2):
                    nc.tensor.matmul(po, ctxt16[:, k, ts(st, 128)],
                                     wo_t16[:, k, ts(nb, 512)],
                                     start=(k == 0), stop=(k == 1))
            else:
                nc.tensor.matmul(po, ctxt[:, :, ts(st, 128)], wo_t[:, :, ts(nb, 512)],
                                 start=True, stop=True, perf_mode=DROW)
            ot = op.tile([128, 512], mybir.dt.float16, tag="ot", name="ot")
            nc.vector.tensor_copy(ot, po)
            nc.sync.dma_start(out=out[ts(st, 128), ts(nb, 512)], in_=ot)

        filler = []

        def emit_filler(n):
            for _ in range(n):
                if filler:
                    filler.pop(0)()

        # ---- prologue ----
        # ~5us of dummy matmuls: spans the DMA wait, flips HAM to full clock
        wps = sc_ps.tile([128, 1024], f32, tag="sc", name="wps")
        for _ in range(12):
            nc.tensor.matmul(wps[:, 0:512], warm[:, 0:128], warm,
                             start=True, stop=True)
        # DMA issue order == global service order == first-needed-first:
        # wq, qh0, qh1, wk, kh0, kh1, wv, vh0, vh1, then sb1; wo on the
        # lightly-used scalar queue so it doesn't delay the stream rails
        def _stile(name, half, sb):
            return sp.tile([128, 4, 512], SDT[name], tag=f"{name}s{half}",
                           name=f"{name}s{half}_{sb}", bufs=3)

        srcs = {"q": qsrc, "k": ksrc}
        t0 = {(n, h): _stile(n, h, 0) for n in "qk" for h in range(2)}
        for h in range(2):
            t0[("v", h)] = v_stream_tile(0, h)[0]
        sstreams[0] = t0
        nc.sync.dma_start(out=wq_t, in_=wq.rearrange("(c p) m -> p c m", p=128).bitcast(DT8))
        for i, (n, h) in enumerate((("q", 0), ("q", 1), ("k", 0), ("k", 1),
                                    ("v", 0), ("v", 1))):
            eng = nc.gpsimd if i % 2 == 0 else nc.sync
            if n == "v":
                src = vsrc16[:, ds(h * 4, 4), :].bitcast(DT)
            else:
                src = srcs[n][:, ds(h * 4, 4), ts(0, 512)].bitcast(SDT[n])
            eng.dma_start(out=t0[(n, h)], in_=src)
            if (n, h) == ("q", 1):
                nc.gpsimd.dma_start(
                    out=wk_t, in_=wk.rearrange("(c p) m -> p c m", p=128).bitcast(DT8))
            if (n, h) == ("k", 1):
                nc.sync.dma_start(
                    out=wv_t, in_=wv.rearrange("(c p) m -> p c m", p=128).bitcast(DT))
                nc.sync.dma_start(
                    out=wv_t8, in_=wv8.rearrange("(c p) m -> p c m", p=128))
        emit_stream_dma(1)
        # one-time broadcast of the V bias for the fused psv drain-add
        bv_bc = pp.tile([128, HPC * DH], DT, name="bv_bc")
        nc.gpsimd.partition_broadcast(bv_bc, bv_t, channels=128)
        psq_unit(0, 0)
        psk_unit(0, 0)

        # ---- main interleaved loop ----
        DEPTH = 4  # attnV deferral depth (pairs)
        for sb in range(NSB):
            if sb < NSB - 2:
                emit_stream_dma(sb + 2)
            if sb == 1:
                # wo loads deferred here so they don't delay the early streams
                nc.gpsimd.dma_start(out=wo_t, in_=wo.rearrange("h p n -> p h n"))
                nc.gpsimd.dma_start(
                    out=wo_t16, in_=wo16.rearrange("h p n -> p h n").bitcast(DT))
            pos = [(st, nb) for st in range(4 * (sb - 1), 4 * sb)
                   for nb in range(2)] if sb >= 1 else []
            for hp in range(2):
                # refill filler queue for this hp
                if sb == 0 and hp == 0:
                    filler.extend([lambda t=t: psv_unit(0, t) for t in range(4)])
                    filler.extend([lambda: psq_unit(0, 1), lambda: psk_unit(0, 1)])
                    filler.extend(
                        [lambda: psq_unit(1, 0), lambda: psk_unit(1, 0),
                         lambda: psq_unit(1, 1), lambda: psk_unit(1, 1)])
                elif hp == 0:
                    filler.extend(
                        (lambda st=st, nb=nb: po_unit(st, nb)) for st, nb in pos[:4])
                    if sb < NSB - 1:
                        nsb = sb + 1
                        filler.extend(
                            [lambda s=nsb: psq_unit(s, 0), lambda s=nsb: psk_unit(s, 0),
                             lambda s=nsb: psq_unit(s, 1), lambda s=nsb: psk_unit(s, 1)])
                if hp == 1:
                    filler.extend(
                        (lambda st=st, nb=nb: po_unit(st, nb)) for st, nb in pos[4:])
                    if sb < NSB - 1:
                        nsb = sb + 1
                        filler.extend(
                            lambda s=nsb, t=t: psv_unit(s, t) for t in range(4))

                jts = list(range(4 * sb + 4))
                nj = len(jts)
                cps = [ctx_ps.tile([HB, 512], f32, tag=f"ctx{a}", name=f"cps{a}")
                       for a in range(2)]
                pend = []
                cur = None

                def lo_of(j):
                    return max(0, (j - 4 * sb) * 128)

                def emit_attnv(pj0, pat, last):
                    # fp8 DoubleRow over the jt pair (pj0, pj0+1): 256-deep
                    # key contraction in one pass
                    lo = lo_of(pj0)
                    for a in range(2):
                        h = 2 * hp + a
                        nc.tensor.matmul(
                            cps[a][:, ds(lo, 512 - lo)],
                            vaug[:, ds(pj0, 2), ds(h * HB, HB)],
                            pat[:, :, ds(a * 512 + lo, 512 - lo)],
                            start=(pj0 == 0), stop=last, perf_mode=DROW)

                def emit_attnv16(pjt, pat, last):
                    # f16 single-jt path for the first query block
                    lo = lo_of(pjt)
                    for a in range(2):
                        h = 2 * hp + a
                        nc.tensor.matmul(
                            cps[a][0:DH + 1, ds(lo, 512 - lo)],
                            vaug16[:, pjt, ds(h * 65, DH + 1)],
                            pat[:, ds(a * 512 + lo, 512 - lo)],
                            start=(pjt == 0), stop=last)

                emit_av = emit_attnv16 if sb == 0 else emit_attnv

                for ji, jt in enumerate(jts):
                    if ji < 2:
                        emit_filler(1)
                    else:
                        left = nj - ji
                        n = max(1, (len(filler) + left - 1) // left) if filler else 0
                        emit_filler(n)
                    straddle = jt >= 4 * sb
                    lo = lo_of(jt)
                    sc = sc_ps.tile([128, 1024], f32, tag="sc")
                    for a in range(2):
                        nc.tensor.matmul(
                            sc[:, ds(a * 512 + lo, 512 - lo)],
                            k2t[hp][ds(a * 64, 64), ts(jt, 128)],
                            q2t[hp][ds(a * 64, 64), ds(sb * 512 + lo, 512 - lo)],
                            start=True, stop=True,
                            tile_position=(a * 64, 0))
                    if straddle:
                        # causal mask: add -60000 over the diagonal triangle
                        # (both heads in one strided op)
                        scv = sc.rearrange("p (a n) -> p a n", a=2)[:, :, ds(lo, 128)]
                        nc.vector.tensor_add(scv, scv, mtri_t)
                    if sb == 0:
                        at = ap.tile([128, 1024], DT, tag="at0", name="at0",
                                     bufs=4)
                        if lo == 0:
                            nc.scalar.activation(at, sc, EXP, scale=0.125)
                        else:
                            nc.scalar.activation(
                                at.rearrange("p (a n) -> p a n", a=2)
                                [:, :, ds(lo, 512 - lo)],
                                sc.rearrange("p (a n) -> p a n", a=2)
                                [:, :, ds(lo, 512 - lo)],
                                EXP, scale=0.125)
                        pend.append((jt, at))
                        if len(pend) > DEPTH:
                            j0, a0 = pend.pop(0)
                            emit_av(j0, a0, False)
                        continue
                    m = ji % 2
                    if m == 0:
                        cur = ap.tile([128, 2, 1024], DT8, tag="atf",
                                      name="atf", bufs=6)
                        if straddle:
                            # zero the pair partner's causally-dead columns on
                            # the (idle) pool engine so they contribute nothing
                            # to the DoubleRow attnV contraction
                            z0 = lo  # partner's dead region is [lo, lo+128)
                            for a in range(2):
                                nc.gpsimd.memset(
                                    cur[:, 1, ds(a * 512 + z0, 128)], 0.0)
                    # terminal window: the tensor drain would otherwise wait on
                    # the ACT backlog; emit a few exps as a DVE int8 Schraudolph
                    # whose bits ARE the fp8e4m3 result
                    fastexp = (sb == NSB - 1 and hp == 1 and 2 <= jt <= 7)
                    if fastexp:
                        # int8 convert truncates toward zero (no rounding), so
                        # bias the Schraudolph constant by +0.5
                        nc.vector.tensor_scalar(
                            cur[:, m, :].bitcast(mybir.dt.int8), sc,
                            0.125 * 8.0 / 0.6931471805599453, 56.15,
                            op0=mybir.AluOpType.mult, op1=mybir.AluOpType.add)
                    elif lo == 0:
                        nc.scalar.activation(cur[:, m, :], sc, EXP, scale=0.125)
                    else:
                        nc.scalar.activation(
                            cur[:, m, :].rearrange("p (a n) -> p a n", a=2)
                            [:, :, ds(lo, 512 - lo)],
                            sc.rearrange("p (a n) -> p a n", a=2)
                            [:, :, ds(lo, 512 - lo)],
                            EXP, scale=0.125)
                    if m == 1:
                        pend.append((jt - 1, cur))
                        if len(pend) > DEPTH:
                            j0, a0 = pend.pop(0)
                            emit_av(j0, a0, False)
                while pend:
                    j0, a0 = pend.pop(0)
                    emit_av(j0, a0, not pend)
                # softmax normalization: 1/denom broadcast-multiplied into ctxt
                for a in range(2):
                    # copy denom to SBUF first: reciprocal_approx_fast is a
                    # bitwise trick and must not read the PSUM port directly
                    dn0 = smp.tile([1, 512], f32, tag="dn0", name="dn0")
                    nc.vector.tensor_copy(dn0, cps[a][ds(DH, 1), :])
                    rd = smp.tile([1, 512], f32, tag="rd", name="rd")
                    nc.vector.reciprocal_approx_fast(rd, dn0)
                    bc = smp.tile([DH, 512], f32, tag="bc", name="bc")
                    nc.gpsimd.partition_broadcast(bc, rd, channels=DH)
                    cdst = (ctxt16[ds(a * DH, DH), hp, :] if sb == 0 else
                            ctxt[ds(a * DH, DH), hp, ts(sb, 512)])
                    nc.vector.tensor_mul(cdst, cps[a][0:DH, :], bc)

        # drain remaining fillers + final output projection
        emit_filler(len(filler))
        for st in range(4 * (NSB - 1), 4 * NSB):
            for nb in range(2):
                po_unit(st, nb)

        cx.close()

    nc.finalize()
    _built[key] = nc
    return nc


def _is_causal(masked: np.ndarray) -> bool:
    c = ~np.tril(np.ones((S, S), dtype=bool))
    return all(np.array_equal(masked[b], c) for b in range(masked.shape[0]))


def kernel(Q, K, V, masked, WQ_w, WQ_b, WK_w, WK_b, WV_w, WV_b, Wo_w, Wo_b):
    global LAST_RESULT
    from concourse.bass_utils import run_bass_kernel_spmd

    Q = np.asarray(Q, dtype=np.float32)
    K = np.asarray(K, dtype=np.float32)
    V = np.asarray(V, dtype=np.float32)
    masked = np.asarray(masked)
    causal = _is_causal(masked)
    if not causal:
        return _kernel_legacy(Q, K, V, masked, WQ_w, WQ_b, WK_w, WK_b,
                              WV_w, WV_b, Wo_w, Wo_b)
    nc = _build_v2(MMDT)
    if MMDT == "f16":
        npdt = np.float16
    elif MMDT == "bf16":
        import ml_dtypes
        npdt = ml_dtypes.bfloat16
    else:
        npdt = np.float32

    import ml_dtypes
    f8 = ml_dtypes.float8_e4m3fn
    qT = [np.ascontiguousarray(Q[b].T.astype(f8)) for b in range(B)]
    kT = [np.ascontiguousarray(K[b].T.astype(f8)) for b in range(B)]
    vT16 = [np.ascontiguousarray(V[b, :512].T.astype(npdt)) for b in range(B)]
    vT8 = [np.ascontiguousarray(V[b, 512:].T.astype(f8)) for b in range(B)]

    j = np.arange(128)[:, None]
    c = np.arange(128)[None, :]
    mtri_1 = ((j > c) * np.float32(NEG)).astype(npdt)
    mtri_full = np.ascontiguousarray(
        np.broadcast_to(mtri_1[:, None, :], (128, 2, 128)))

    in_maps = []
    for cc in range(NCORES):
        b = cc // CORES_PER_BATCH
        h0 = (cc % CORES_PER_BATCH) * HPC
        sel = slice(h0 * DH, (h0 + HPC) * DH)
        wo_pad = np.asarray(Wo_w).T[sel].reshape(2, 128, D).astype(np.float32)
        m = {
            "qt": qT[b], "kt": kT[b], "vt16": vT16[b], "vt8": vT8[b],
            "wq": np.ascontiguousarray(np.asarray(WQ_w)[sel].T.astype(f8)),
            "wk": np.ascontiguousarray(np.asarray(WK_w)[sel].T.astype(f8)),
            "wv": np.ascontiguousarray(np.asarray(WV_w)[sel].T.astype(npdt)),
            "wv8": np.ascontiguousarray(np.asarray(WV_w)[sel].T.astype(f8)),
            "wo": wo_pad.astype(f8),
            "wo16": wo_pad.astype(npdt),
            "bq": np.ascontiguousarray(np.asarray(WQ_b)[sel].reshape(2, 128).T.astype(np.float32)),
            "bk": np.ascontiguousarray(np.asarray(WK_b)[sel].reshape(2, 128).T.astype(np.float32)),
            "bv": np.ascontiguousarray(np.asarray(WV_b)[sel].reshape(1, HPC * DH).astype(npdt)),
            "mtri": mtri_full,
        }
        m = {k: np.ascontiguousarray(v) for k, v in m.items()}
        in_maps.append(m)

    res = run_bass_kernel_spmd(nc, in_maps, core_ids=list(range(NCORES)), trace=TRACE)
    LAST_RESULT = res

    acc = np.zeros((B, S, D), np.float64)
    for cc in range(NCORES):
        acc[cc // CORES_PER_BATCH] += res.results[cc]["out"].astype(np.float64)
    acc += np.asarray(Wo_b, dtype=np.float64)[None, None, :]
    return acc.astype(np.float32)


# ---------------------------------------------------------------------------
# legacy non-causal fallback (general mask multiply path)
# ---------------------------------------------------------------------------

def _build_legacy(mmdt: str):
    key = ("legacy", mmdt)
    if key in _built:
        return _built[key]
    import concourse.mybir as mybir
    import concourse.tile as tile
    from concourse import bacc
    from concourse.bass import ts, ds

    f32 = mybir.dt.float32
    DT = {"f32r": mybir.dt.float32r, "f16": mybir.dt.float16,
          "bf16": mybir.dt.bfloat16}[mmdt]
    DTNP = {"f32r": f32, "f16": mybir.dt.float16, "bf16": mybir.dt.bfloat16}[mmdt]
    EXP = mybir.ActivationFunctionType.Exp

    nc = bacc.Bacc("TRN2")
    qt = nc.dram_tensor("qt", [D, S], DTNP, kind="ExternalInput")
    kt = nc.dram_tensor("kt", [D, S], DTNP, kind="ExternalInput")
    vt = nc.dram_tensor("vt", [D, S], DTNP, kind="ExternalInput")
    wq = nc.dram_tensor("wq", [D, HPC * DH], DTNP, kind="ExternalInput")
    wk = nc.dram_tensor("wk", [D, HPC * DH], DTNP, kind="ExternalInput")
    wv = nc.dram_tensor("wv", [D, HPC * DH], DTNP, kind="ExternalInput")
    wo = nc.dram_tensor("wo", [2, 128, D], DTNP, kind="ExternalInput")
    bq = nc.dram_tensor("bq", [128, 2], f32, kind="ExternalInput")
    bk = nc.dram_tensor("bk", [128, 2], f32, kind="ExternalInput")
    bv = nc.dram_tensor("bv", [1, HPC * DH], DTNP, kind="ExternalInput")
    mt = nc.dram_tensor("mt", [S, S], DTNP, kind="ExternalInput")
    out = nc.dram_tensor("out", [S, D], f32, kind="ExternalOutput")

    NSB = S // 512
    NST = S // 128

    import contextlib
    with tile.TileContext(nc) as tc, contextlib.ExitStack() as ctx_pools:
        with (
            tc.tile_pool(name="persist", bufs=1) as pp,
            tc.tile_pool(name="sc_ps", bufs=2, space="PSUM") as sc_ps,
            tc.tile_pool(name="ctx_ps", bufs=1, space="PSUM") as ctx_ps,
            tc.tile_pool(name="mm_ps", bufs=2, space="PSUM") as mm_ps,
        ):
            bq_t = pp.tile([128, 2], f32)
            nc.gpsimd.dma_start(out=bq_t, in_=bq[:, :])
            bk_t = pp.tile([128, 2], f32)
            nc.gpsimd.dma_start(out=bk_t, in_=bk[:, :])
            bv_t = pp.tile([1, HPC * DH], DT)
            nc.gpsimd.dma_start(out=bv_t, in_=bv[:, :].bitcast(DT))

            ones_c = pp.tile([1, 128], DT)
            nc.vector.memset(ones_c.bitcast(DTNP), 1.0)

            ap = ctx_pools.enter_context(tc.tile_pool(name="attn", bufs=3))
            smp = ctx_pools.enter_context(tc.tile_pool(name="small", bufs=3))
            mlp = ctx_pools.enter_context(tc.tile_pool(name="mload", bufs=3))
            owp = ctx_pools.enter_context(tc.tile_pool(name="outw", bufs=1))
            op = ctx_pools.enter_context(tc.tile_pool(name="outp", bufs=4))

            q2t = [pp.tile([128, S], DT, tag=f"q2t{i}", name=f"q2t{i}") for i in range(2)]
            k2t = [pp.tile([128, S], DT, tag=f"k2t{i}", name=f"k2t{i}") for i in range(2)]
            vaug = pp.tile([128, NST, HPC * (DH + 1)], DT)
            for h in range(HPC):
                nc.vector.memset(vaug[:, :, ds(h * 65 + 64, 1)].bitcast(DTNP), 1.0)
            ctxt = [pp.tile([128, S], DT, tag=f"ctxt{i}", name=f"ctxt{i}") for i in range(2)]

            with (
                tc.tile_pool(name="wproj", bufs=1) as wp,
                tc.tile_pool(name="stream", bufs=2) as sp,
            ):
                wq_t = wp.tile([128, 8, 256], DT)
                nc.gpsimd.dma_start(out=wq_t, in_=wq.rearrange("(c p) m -> p c m", p=128).bitcast(DT))
                wk_t = wp.tile([128, 8, 256], DT)
                nc.gpsimd.dma_start(out=wk_t, in_=wk.rearrange("(c p) m -> p c m", p=128).bitcast(DT))
                wv_t = wp.tile([128, 8, 256], DT)
                nc.gpsimd.dma_start(out=wv_t, in_=wv.rearrange("(c p) m -> p c m", p=128).bitcast(DT))

                for sb in range(NSB):
                    qs = [sp.tile([128, 4, 512], DT, tag=f"qs{i}", name=f"qs{i}", bufs=1) for i in range(2)]
                    ks = [sp.tile([128, 4, 512], DT, tag=f"ks{i}", name=f"ks{i}", bufs=1) for i in range(2)]
                    vs = [sp.tile([128, 4, 512], DT, tag=f"vs{i}", name=f"vs{i}", bufs=1) for i in range(2)]
                    for half in range(2):
                        for name, t, dr, eng in (("q", qs, qt, nc.sync),
                                                 ("k", ks, kt, nc.sync),
                                                 ("v", vs, vt, nc.gpsimd)):
                            src = dr.rearrange("(c p) s -> p c s", p=128)
                            eng.dma_start(
                                out=t[half],
                                in_=src[:, ds(half * 4, 4), ts(sb, 512)].bitcast(DT),
                            )
                    for hp in range(2):
                        psq = mm_ps.tile([128, 512], f32, tag="mm")
                        for c in range(8):
                            nc.tensor.matmul(
                                psq, wq_t[:, c, ts(hp, 128)], qs[c // 4][:, c % 4, :],
                                start=(c == 0), stop=(c == 7),
                            )
                        nc.vector.tensor_scalar_add(
                            q2t[hp][:, ts(sb, 512)], psq, bq_t[:, ds(hp, 1)])
                        psk = mm_ps.tile([128, 512], f32, tag="mm")
                        for c in range(8):
                            nc.tensor.matmul(
                                psk, wk_t[:, c, ts(hp, 128)], ks[c // 4][:, c % 4, :],
                                start=(c == 0), stop=(c == 7),
                            )
                        nc.vector.tensor_scalar_add(
                            k2t[hp][:, ts(sb, 512)], psk, bk_t[:, ds(hp, 1)])
                    for st4 in range(4):
                        st = sb * 4 + st4
                        psv = mm_ps.tile([128, 256], f32, tag="mm")
                        for c in range(8):
                            nc.tensor.matmul(
                                psv, vs[c // 4][:, c % 4, ts(st4, 128)], wv_t[:, c, :],
                                start=(c == 0), stop=False,
                            )
                        nc.tensor.matmul(psv, ones_c, bv_t, start=False, stop=True)
                        nc.vector.tensor_copy(
                            vaug[:, st, :].rearrange("p (h x) -> p h x", h=HPC)[:, :, 0:DH],
                            psv.rearrange("p (h x) -> p h x", h=HPC),
                        )

            wo_t = owp.tile([128, 2, D], DT)
            nc.gpsimd.dma_start(out=wo_t, in_=wo.rearrange("h p n -> p h n").bitcast(DT))
            outq = []

            def emit_out_unit():
                if not outq:
                    return
                st, nb = outq.pop(0)
                po = mm_ps.tile([128, 512], f32, tag="mm", name="po")
                for k in range(2):
                    nc.tensor.matmul(
                        po, ctxt[k][:, ts(st, 128)],
                        wo_t[:, k, ts(nb, 512)],
                        start=(k == 0), stop=(k == 1),
                    )
                ot = op.tile([128, 512], f32, tag="ot", name="ot")
                nc.any.tensor_copy(ot, po)
                (nc.gpsimd if (st + nb) % 2 else nc.sync).dma_start(out=out[ts(st, 128), ts(nb, 512)], in_=ot)

            for sb in range(NSB):
                for hp in range(2):
                    jts = list(range(NST))
                    cps = [ctx_ps.tile([DH + 1, 512], f32, tag=f"ctx{a}", name=f"cps{a}")
                           for a in range(2)]
                    pending = None

                    def emit_attnv(pjt, pat, last):
                        for a in range(2):
                            h = 2 * hp + a
                            nc.tensor.matmul(
                                cps[a],
                                vaug[:, pjt, ds(h * 65, DH + 1)],
                                pat[:, ds(a * 512, 512)],
                                start=(pjt == jts[0]), stop=last,
                            )

                    for jt in jts:
                        emit_out_unit()
                        sc = sc_ps.tile([128, 1024], f32, tag="sc")
                        mt_t = mlp.tile([128, 512], DT, tag="mt")
                        nc.sync.dma_start(
                            out=mt_t,
                            in_=mt[ts(jt, 128), ts(sb, 512)].bitcast(DT))
                        for a in range(2):
                            nc.tensor.matmul(
                                sc[:, ds(a * 512, 512)],
                                k2t[hp][ds(a * 64, 64), ts(jt, 128)],
                                q2t[hp][ds(a * 64, 64), ds(sb * 512, 512)],
                                start=True, stop=True,
                                tile_position=(a * 64, 0),
                            )
                        at = ap.tile([128, 1024], DT, tag="at")
                        nc.scalar.activation(at, sc, EXP, scale=0.125)
                        for a in range(2):
                            nc.vector.tensor_mul(
                                at[:, ts(a, 512)], at[:, ts(a, 512)], mt_t)
                        if pending is not None:
                            emit_attnv(pending[0], pending[1], False)
                        pending = (jt, at)
                    emit_attnv(pending[0], pending[1], True)
                    for a in range(2):
                        dn0 = smp.tile([1, 512], f32, tag="dn0", name="dn0")
                        nc.vector.tensor_copy(dn0, cps[a][ds(DH, 1), :])
                        rd = smp.tile([1, 512], f32, tag="rd", name="rd")
                        nc.vector.reciprocal_approx_fast(rd, dn0)
                        bc = smp.tile([DH, 512], f32, tag="bc", name="bc")
                        nc.gpsimd.partition_broadcast(bc, rd, channels=DH)
                        nc.vector.tensor_mul(
                            ctxt[hp][ds(a * DH, DH), ts(sb, 512)],
                            cps[a][0:DH, :], bc)
                outq.extend((st, nb) for st in range(4 * sb, 4 * sb + 4)
                            for nb in range(2))
            while outq:
                emit_out_unit()

            ctx_pools.close()

    nc.finalize()
    _built[key] = nc
    return nc


def _kernel_legacy(Q, K, V, masked, WQ_w, WQ_b, WK_w, WK_b, WV_w, WV_b, Wo_w, Wo_b):
    global LAST_RESULT
    from concourse.bass_utils import run_bass_kernel_spmd

    nc = _build_legacy(MMDT)
    if MMDT == "f16":
        npdt = np.float16
    elif MMDT == "bf16":
        import ml_dtypes
        npdt = ml_dtypes.bfloat16
    else:
        npdt = np.float32

    qT = [np.ascontiguousarray(Q[b].T.astype(npdt)) for b in range(B)]
    kT = [np.ascontiguousarray(K[b].T.astype(npdt)) for b in range(B)]
    vT = [np.ascontiguousarray(V[b].T.astype(npdt)) for b in range(B)]
    mtb = [np.ascontiguousarray(
        np.where(masked[b].T, np.float32(0.0), np.float32(1.0)).astype(npdt))
        for b in range(B)]

    in_maps = []
    for c in range(NCORES):
        b = c // CORES_PER_BATCH
        h0 = (c % CORES_PER_BATCH) * HPC
        sel = slice(h0 * DH, (h0 + HPC) * DH)
        wo_pad = np.asarray(Wo_w).T[sel].reshape(2, 128, D).astype(np.float32)
        m = {
            "qt": qT[b], "kt": kT[b], "vt": vT[b],
            "wq": np.ascontiguousarray(np.asarray(WQ_w)[sel].T.astype(npdt)),
            "wk": np.ascontiguousarray(np.asarray(WK_w)[sel].T.astype(npdt)),
            "wv": np.ascontiguousarray(np.asarray(WV_w)[sel].T.astype(npdt)),
            "wo": wo_pad.astype(npdt),
            "bq": np.ascontiguousarray(np.asarray(WQ_b)[sel].reshape(2, 128).T.astype(np.float32)),
            "bk": np.ascontiguousarray(np.asarray(WK_b)[sel].reshape(2, 128).T.astype(np.float32)),
            "bv": np.ascontiguousarray(np.asarray(WV_b)[sel].reshape(1, HPC * DH).astype(npdt)),
            "mt": mtb[b],
        }
        m = {k: np.ascontiguousarray(v) for k, v in m.items()}
        in_maps.append(m)

    res = run_bass_kernel_spmd(nc, in_maps, core_ids=list(range(NCORES)), trace=TRACE)
    LAST_RESULT = res

    acc = np.zeros((B, S, D), np.float64)
    for c in range(NCORES):
        acc[c // CORES_PER_BATCH] += res.results[c]["out"].astype(np.float64)
    acc += np.asarray(Wo_b, dtype=np.float64)[None, None, :]
    return acc.astype(np.float32)



# revision 69
# speedup vs baseline: 1.4199x; 1.1305x over previous
"""Multi-head attention (B=2, S=2048, D=1024, H=16) on 8 TRN2 NeuronCores.

Sharding: batch (2) x head-groups (4 heads/core). Each core computes its
batch's QKV projections for its 4 heads, causal attention, and a partial
output projection over its head slice; the host sums the 4 partials per
batch and adds the output bias.

Layout strategy: everything runs in "transposed" orientation so no on-chip
transposes are needed:
  q2^T[dm, s] = Wq[dm,:] @ Q^T       (host supplies Q^T and Wq^T)
  scores^T[j, si] = k^T.T @ q^T      (d_h contraction, 2 heads row-tiled)
  attn^T = exp(scores^T/8 + mask)    (mask added pre-exp via -60000*tri matmul)
  ctx^T+denom = [v | 1].T @ attn^T   (ones column gives softmax denominator)
  out[s, n] = ctxn^T.T @ Wo^T        (K=128 chunks x2)
v2: single interleaved pipeline -- QKV projection and output projection
units are emitted as fillers inside the attention jt loop so the tensor
queue never drains (keeps HAM at full clock); mask applied pre-exp on the
tensor engine; straddle exp in one strided-AP instruction; f16 output.
"""

import numpy as np

B, S, D, H, DH = 2, 2048, 1024, 16, 64
NCORES = 8
CORES_PER_BATCH = 4
HPC = H // CORES_PER_BATCH  # heads per core = 4
NEG = -60000.0  # exp((x+NEG)/8) == 0 exactly in fp32; fits in fp16
MMDT = "f16"

TRACE = False  # test.py sets True to collect an NTFF profile
LAST_RESULT = None  # BassKernelResults from the last run (for test.py)

_built = {}


def _build_v2(mmdt: str):
    key = ("v2", mmdt)
    if key in _built:
        return _built[key]
    import concourse.mybir as mybir
    import concourse.tile as tile
    from concourse import bacc
    from concourse.bass import ts, ds

    f32 = mybir.dt.float32
    DT = {"f32r": mybir.dt.float32r, "f16": mybir.dt.float16,
          "bf16": mybir.dt.bfloat16}[mmdt]
    DTNP = {"f32r": f32, "f16": mybir.dt.float16, "bf16": mybir.dt.bfloat16}[mmdt]
    EXP = mybir.ActivationFunctionType.Exp
    DT8 = mybir.dt.float8e4
    DROW = mybir.MatmulPerfMode.DoubleRow

    nc = bacc.Bacc("TRN2")
    qt = nc.dram_tensor("qt", [D, S], DT8, kind="ExternalInput")
    kt = nc.dram_tensor("kt", [D, S], DT8, kind="ExternalInput")
    # V split by key range: keys < 512 stay f16 (feed the exact early-query
    # path); keys >= 512 are fp8 (only ever read by spread-attention queries)
    vt16 = nc.dram_tensor("vt16", [D, 512], DTNP, kind="ExternalInput")
    vt8 = nc.dram_tensor("vt8", [D, S - 512], DT8, kind="ExternalInput")
    wq = nc.dram_tensor("wq", [D, HPC * DH], DT8, kind="ExternalInput")
    wk = nc.dram_tensor("wk", [D, HPC * DH], DT8, kind="ExternalInput")
    wv = nc.dram_tensor("wv", [D, HPC * DH], DTNP, kind="ExternalInput")
    wv8 = nc.dram_tensor("wv8", [D, HPC * DH], DT8, kind="ExternalInput")
    wo = nc.dram_tensor("wo", [2, 128, D], DT8, kind="ExternalInput")
    wo16 = nc.dram_tensor("wo16", [2, 128, D], DTNP, kind="ExternalInput")
    bq = nc.dram_tensor("bq", [128, 2], f32, kind="ExternalInput")
    bk = nc.dram_tensor("bk", [128, 2], f32, kind="ExternalInput")
    bv = nc.dram_tensor("bv", [1, HPC * DH], DTNP, kind="ExternalInput")
    mtri = nc.dram_tensor("mtri", [128, 2, 128], DTNP, kind="ExternalInput")
    out = nc.dram_tensor("out", [S, D], mybir.dt.float16, kind="ExternalOutput")

    NSB = S // 512   # 4 si-blocks of 512
    NST = S // 128   # 16 s-tiles / j-tiles of 128

    import contextlib
    with tile.TileContext(nc) as tc, contextlib.ExitStack() as cx:
        pp = cx.enter_context(tc.tile_pool(name="persist", bufs=1))
        sc_ps = cx.enter_context(tc.tile_pool(name="sc_ps", bufs=2, space="PSUM"))
        ctx_ps = cx.enter_context(tc.tile_pool(name="ctx_ps", bufs=1, space="PSUM"))
        mm_ps = cx.enter_context(tc.tile_pool(name="mm_ps", bufs=2, space="PSUM"))
        sp = cx.enter_context(tc.tile_pool(name="stream", bufs=1))
        ap = cx.enter_context(tc.tile_pool(name="attn", bufs=4))
        smp = cx.enter_context(tc.tile_pool(name="small", bufs=3))
        op = cx.enter_context(tc.tile_pool(name="outp", bufs=4))

        # ---- persistent tiles + constant DMAs (small, on scalar queue) ----
        bq_t = pp.tile([128, 2], f32)
        nc.scalar.dma_start(out=bq_t, in_=bq[:, :])
        bk_t = pp.tile([128, 2], f32)
        nc.scalar.dma_start(out=bk_t, in_=bk[:, :])
        bv_t = pp.tile([1, HPC * DH], DT)
        nc.scalar.dma_start(out=bv_t, in_=bv[:, :].bitcast(DT))
        mtri_t = pp.tile([128, 2, 128], DT)
        nc.scalar.dma_start(out=mtri_t, in_=mtri[:, :, :].bitcast(DT))

        ones_c = pp.tile([1, 128], DT)
        nc.vector.memset(ones_c.bitcast(DTNP), 1.0)
        warm = pp.tile([128, 512], DT, name="warm")
        nc.vector.memset(warm.bitcast(DTNP), 1.0)

        q2t = [pp.tile([128, S], DT, tag=f"q2t{i}", name=f"q2t{i}") for i in range(2)]
        k2t = [pp.tile([128, S], DT, tag=f"k2t{i}", name=f"k2t{i}") for i in range(2)]
        # fp8 vaug in 96-wide head blocks (v 0:64 | ones 64 | zero pad 65:96):
        # dual-fp8 ldweights requires a 32-multiple stationary free size
        HB = 96
        vaug = pp.tile([128, NST, HPC * HB], DT8)
        for h in range(HPC):
            nc.vector.memset(vaug[:, :, ds(h * HB + 64, 1)], 1.0)
            nc.vector.memset(vaug[:, :, ds(h * HB + 65, HB - 65)], 0.0)
        # f16 copies for the first query block (sb=0): near-one-hot attention
        # there makes ctx ~ a raw V row, so fp8 V/ctx/Wo would be ~4% off
        vaug16 = pp.tile([128, 4, HPC * (DH + 1)], DT, name="vaug16")
        for h in range(HPC):
            nc.vector.memset(vaug16[:, :, ds(h * 65 + 64, 1)].bitcast(DTNP), 1.0)
        ctxt = pp.tile([128, 2, S], DT8, name="ctxt")
        ctxt16 = pp.tile([128, 2, 512], DT, name="ctxt16")

        wq_t = pp.tile([128, 8, 256], DT8)
        wk_t = pp.tile([128, 8, 256], DT8)
        wv_t = pp.tile([128, 8, 256], DT)
        wv_t8 = pp.tile([128, 8, 256], DT8)
        wo_t = pp.tile([128, 2, D], DT8)
        wo_t16 = pp.tile([128, 2, D], DT)

        # ---- stream tiles: [128, 4, 512] halves, triple buffered over sb so
        # the sb+2 prefetch DMA can start before sb's reads finish ----
        qsrc = qt.rearrange("(c p) s -> p c s", p=128)
        ksrc = kt.rearrange("(c p) s -> p c s", p=128)
        vsrc16 = vt16.rearrange("(c p) s -> p c s", p=128)
        vsrc8 = vt8.rearrange("(c p) s -> p c s", p=128)
        sstreams = {}

        SDT = {"q": DT8, "k": DT8, "v": DT}

        def v_stream_tile(sb, half):
            if sb == 0:
                tl = sp.tile([128, 4, 512], DT, tag=f"v16s{half}",
                             name=f"v16s{half}", bufs=1)
                src = vsrc16[:, ds(half * 4, 4), :].bitcast(DT)
            else:
                tl = sp.tile([128, 4, 512], DT8, tag=f"vs{half}",
                             name=f"vs{half}_{sb}", bufs=3)
                src = vsrc8[:, ds(half * 4, 4), ts(sb - 1, 512)]
            return tl, src

        def emit_stream_dma(sb):
            t = {}
            for name, src in (("q", qsrc), ("k", ksrc)):
                for half in range(2):
                    tl = sp.tile([128, 4, 512], SDT[name], tag=f"{name}s{half}",
                                 name=f"{name}s{half}_{sb}", bufs=3)
                    (nc.sync if half == 0 else nc.gpsimd).dma_start(
                        out=tl, in_=src[:, ds(half * 4, 4), ts(sb, 512)].bitcast(SDT[name]))
                    t[(name, half)] = tl
            for half in range(2):
                tl, src = v_stream_tile(sb, half)
                (nc.sync if half == 0 else nc.gpsimd).dma_start(out=tl, in_=src)
                t[("v", half)] = tl
            sstreams[sb] = t

        def chunk(sb, name, c):
            return sstreams[sb][(name, c // 4)][:, c % 4, :]

        # ---- compute units ----
        def chunk2(sb, name, j):
            # adjacent chunk pair (2j, 2j+1) as a [128, 2, 512] AP for DoubleRow
            half, r = divmod(2 * j, 4)
            return sstreams[sb][(name, half)][:, ds(r, 2), :]

        def psq_unit(sb, hp):
            ps = mm_ps.tile([128, 512], f32, tag="mm", name="psq")
            for j in range(4):
                nc.tensor.matmul(ps, wq_t[:, ds(2 * j, 2), ts(hp, 128)],
                                 chunk2(sb, "q", j),
                                 start=(j == 0), stop=(j == 3), perf_mode=DROW)
            nc.vector.tensor_scalar_add(q2t[hp][:, ts(sb, 512)], ps, bq_t[:, ds(hp, 1)])

        def psk_unit(sb, hp):
            ps = mm_ps.tile([128, 512], f32, tag="mm", name="psk")
            for j in range(4):
                nc.tensor.matmul(ps, wk_t[:, ds(2 * j, 2), ts(hp, 128)],
                                 chunk2(sb, "k", j),
                                 start=(j == 0), stop=(j == 3), perf_mode=DROW)
            nc.vector.tensor_scalar_add(k2t[hp][:, ts(sb, 512)], ps, bk_t[:, ds(hp, 1)])

        def psv_unit(sb, st4):
            st = sb * 4 + st4
            ps = mm_ps.tile([128, 256], f32, tag="mm", name="psv")
            if sb == 0:
                for c in range(8):
                    nc.tensor.matmul(ps, chunk(sb, "v", c)[:, ts(st4, 128)],
                                     wv_t[:, c, :], start=(c == 0), stop=(c == 7))
            else:
                for j in range(4):
                    half, r = divmod(2 * j, 4)
                    vpair = sstreams[sb][("v", half)][:, ds(r, 2), ts(st4, 128)]
                    nc.tensor.matmul(ps, vpair, wv_t8[:, ds(2 * j, 2), :],
                                     start=(j == 0), stop=(j == 3), perf_mode=DROW)
            bv3 = bv_bc.rearrange("p (h x) -> p h x", h=HPC)
            nc.vector.tensor_add(
                vaug[:, st, :].rearrange("p (h x) -> p h x", h=HPC)[:, :, 0:DH],
                ps.rearrange("p (h x) -> p h x", h=HPC), bv3)
            if st < 4:
                nc.vector.tensor_add(
                    vaug16[:, st, :].rearrange("p (h x) -> p h x", h=HPC)[:, :, 0:DH],
                    ps.rearrange("p (h x) -> p h x", h=HPC), bv3)

        def po_unit(st, nb):
            po = mm_ps.tile([128, 512], f32, tag="mm", name="po")
            if st < 4:
                for k in range(2):
                    nc.tensor.matmul(po, ctxt16[:, k, ts(st, 128)],
                                     wo_t16[:, k, ts(nb, 512)],
                                     start=(k == 0), stop=(k == 1))
            else:
                nc.tensor.matmul(po, ctxt[:, :, ts(st, 128)], wo_t[:, :, ts(nb, 512)],
                                 start=True, stop=True, perf_mode=DROW)
            ot = op.tile([128, 512], mybir.dt.float16, tag="ot", name="ot")
            nc.vector.tensor_copy(ot, po)
            nc.scalar.dma_start(out=out[ts(st, 128), ts(nb, 512)], in_=ot)

        filler = []

        def emit_filler(n):
            for _ in range(n):
                if filler:
                    filler.pop(0)()

        # ---- prologue ----
        # ~5us of dummy matmuls: spans the DMA wait, flips HAM to full clock
        wps = sc_ps.tile([128, 1024], f32, tag="sc", name="wps")
        for _ in range(12):
            nc.tensor.matmul(wps[:, 0:512], warm[:, 0:128], warm,
                             start=True, stop=True)
        # DMA issue order == global service order == first-needed-first:
        # wq, qh0, qh1, wk, kh0, kh1, wv, vh0, vh1, then sb1; wo on the
        # lightly-used scalar queue so it doesn't delay the stream rails
        def _stile(name, half, sb):
            return sp.tile([128, 4, 512], SDT[name], tag=f"{name}s{half}",
                           name=f"{name}s{half}_{sb}", bufs=3)

        srcs = {"q": qsrc, "k": ksrc}
        t0 = {(n, h): _stile(n, h, 0) for n in "qk" for h in range(2)}
        for h in range(2):
            t0[("v", h)] = v_stream_tile(0, h)[0]
        sstreams[0] = t0
        nc.sync.dma_start(out=wq_t, in_=wq.rearrange("(c p) m -> p c m", p=128).bitcast(DT8))
        for i, (n, h) in enumerate((("q", 0), ("q", 1), ("k", 0), ("k", 1),
                                    ("v", 0), ("v", 1))):
            eng = nc.gpsimd if i % 2 == 0 else nc.sync
            if n == "v":
                src = vsrc16[:, ds(h * 4, 4), :].bitcast(DT)
            else:
                src = srcs[n][:, ds(h * 4, 4), ts(0, 512)].bitcast(SDT[n])
            eng.dma_start(out=t0[(n, h)], in_=src)
            if (n, h) == ("q", 1):
                nc.gpsimd.dma_start(
                    out=wk_t, in_=wk.rearrange("(c p) m -> p c m", p=128).bitcast(DT8))
            if (n, h) == ("k", 1):
                nc.sync.dma_start(
                    out=wv_t, in_=wv.rearrange("(c p) m -> p c m", p=128).bitcast(DT))
                nc.sync.dma_start(
                    out=wv_t8, in_=wv8.rearrange("(c p) m -> p c m", p=128))
        emit_stream_dma(1)
        # one-time broadcast of the V bias for the fused psv drain-add
        bv_bc = pp.tile([128, HPC * DH], DT, name="bv_bc")
        nc.gpsimd.partition_broadcast(bv_bc, bv_t, channels=128)
        psq_unit(0, 0)
        psk_unit(0, 0)

        # ---- main interleaved loop ----
        DEPTH = 3  # attnV deferral depth (pairs)
        for sb in range(NSB):
            if sb < NSB - 2:
                emit_stream_dma(sb + 2)
            if sb == 1:
                # wo loads deferred here so they don't delay the early streams
                nc.gpsimd.dma_start(out=wo_t, in_=wo.rearrange("h p n -> p h n"))
                nc.gpsimd.dma_start(
                    out=wo_t16, in_=wo16.rearrange("h p n -> p h n").bitcast(DT))
            pos = [(st, nb) for st in range(4 * (sb - 1), 4 * sb)
                   for nb in range(2)] if sb >= 1 else []
            for hp in range(2):
                # refill filler queue for this hp
                if sb == 0 and hp == 0:
                    filler.extend([lambda t=t: psv_unit(0, t) for t in range(4)])
                    filler.extend([lambda: psq_unit(0, 1), lambda: psk_unit(0, 1)])
                    filler.extend(
                        [lambda: psq_unit(1, 0), lambda: psk_unit(1, 0),
                         lambda: psq_unit(1, 1), lambda: psk_unit(1, 1)])
                elif hp == 0:
                    filler.extend(
                        (lambda st=st, nb=nb: po_unit(st, nb)) for st, nb in pos[:4])
                    if sb < NSB - 1:
                        nsb = sb + 1
                        filler.extend(
                            [lambda s=nsb: psq_unit(s, 0), lambda s=nsb: psk_unit(s, 0),
                             lambda s=nsb: psq_unit(s, 1), lambda s=nsb: psk_unit(s, 1)])
                if hp == 1:
                    filler.extend(
                        (lambda st=st, nb=nb: po_unit(st, nb)) for st, nb in pos[4:])
                    if sb < NSB - 1:
                        nsb = sb + 1
                        filler.extend(
                            lambda s=nsb, t=t: psv_unit(s, t) for t in range(4))

                jts = list(range(4 * sb + 4))
                nj = len(jts)
                cps = [ctx_ps.tile([HB, 512], f32, tag=f"ctx{a}", name=f"cps{a}")
                       for a in range(2)]
                pend = []
                cur = None

                def lo_of(j):
                    return max(0, (j - 4 * sb) * 128)

                def emit_attnv(pj0, pat, last):
                    # fp8 DoubleRow over the jt pair (pj0, pj0+1): 256-deep
                    # key contraction in one pass
                    lo = lo_of(pj0)
                    for a in range(2):
                        h = 2 * hp + a
                        nc.tensor.matmul(
                            cps[a][:, ds(lo, 512 - lo)],
                            vaug[:, ds(pj0, 2), ds(h * HB, HB)],
                            pat[:, :, ds(a * 512 + lo, 512 - lo)],
                            start=(pj0 == 0), stop=last, perf_mode=DROW)

                def emit_attnv16(pjt, pat, last):
                    # f16 single-jt path for the first query block
                    lo = lo_of(pjt)
                    for a in range(2):
                        h = 2 * hp + a
                        nc.tensor.matmul(
                            cps[a][0:DH + 1, ds(lo, 512 - lo)],
                            vaug16[:, pjt, ds(h * 65, DH + 1)],
                            pat[:, ds(a * 512 + lo, 512 - lo)],
                            start=(pjt == 0), stop=last)

                emit_av = emit_attnv16 if sb == 0 else emit_attnv

                for ji, jt in enumerate(jts):
                    if ji < 2:
                        emit_filler(1)
                    else:
                        left = nj - ji
                        n = max(1, (len(filler) + left - 1) // left) if filler else 0
                        emit_filler(n)
                    straddle = jt >= 4 * sb
                    lo = lo_of(jt)
                    sc = sc_ps.tile([128, 1024], f32, tag="sc")
                    for a in range(2):
                        nc.tensor.matmul(
                            sc[:, ds(a * 512 + lo, 512 - lo)],
                            k2t[hp][ds(a * 64, 64), ts(jt, 128)],
                            q2t[hp][ds(a * 64, 64), ds(sb * 512 + lo, 512 - lo)],
                            start=True, stop=True,
                            tile_position=(a * 64, 0))
                    if straddle:
                        # causal mask: add -60000 over the diagonal triangle
                        # (both heads in one strided op)
                        scv = sc.rearrange("p (a n) -> p a n", a=2)[:, :, ds(lo, 128)]
                        nc.vector.tensor_add(scv, scv, mtri_t)
                    if sb == 0:
                        at = ap.tile([128, 1024], DT, tag="at0", name="at0",
                                     bufs=4)
                        if lo == 0:
                            nc.scalar.activation(at, sc, EXP, scale=0.125)
                        else:
                            nc.scalar.activation(
                                at.rearrange("p (a n) -> p a n", a=2)
                                [:, :, ds(lo, 512 - lo)],
                                sc.rearrange("p (a n) -> p a n", a=2)
                                [:, :, ds(lo, 512 - lo)],
                                EXP, scale=0.125)
                        pend.append((jt, at))
                        if len(pend) > DEPTH:
                            j0, a0 = pend.pop(0)
                            emit_av(j0, a0, False)
                        continue
                    m = ji % 2
                    if m == 0:
                        cur = ap.tile([128, 2, 1024], DT8, tag="atf",
                                      name="atf", bufs=4)
                        if straddle:
                            # zero the pair partner's causally-dead columns on
                            # the (idle) pool engine so they contribute nothing
                            # to the DoubleRow attnV contraction
                            z0 = lo  # partner's dead region is [lo, lo+128)
                            for a in range(2):
                                nc.gpsimd.memset(
                                    cur[:, 1, ds(a * 512 + z0, 128)], 0.0)
                    # terminal window: the tensor drain would otherwise wait on
                    # the ACT backlog; emit a few exps as a DVE int8 Schraudolph
                    # whose bits ARE the fp8e4m3 result
                    fastexp = (sb == NSB - 1 and hp == 1 and 2 <= jt <= 7)
                    if fastexp:
                        # int8 convert truncates toward zero (no rounding), so
                        # bias the Schraudolph constant by +0.5
                        nc.vector.tensor_scalar(
                            cur[:, m, :].bitcast(mybir.dt.int8), sc,
                            0.125 * 8.0 / 0.6931471805599453, 56.15,
                            op0=mybir.AluOpType.mult, op1=mybir.AluOpType.add)
                    elif lo == 0:
                        nc.scalar.activation(cur[:, m, :], sc, EXP, scale=0.125)
                    else:
                        nc.scalar.activation(
                            cur[:, m, :].rearrange("p (a n) -> p a n", a=2)
                            [:, :, ds(lo, 512 - lo)],
                            sc.rearrange("p (a n) -> p a n", a=2)
                            [:, :, ds(lo, 512 - lo)],
                            EXP, scale=0.125)
                    if m == 1:
                        pend.append((jt - 1, cur))
                        if len(pend) > DEPTH:
                            j0, a0 = pend.pop(0)
                            emit_av(j0, a0, False)
                while pend:
                    j0, a0 = pend.pop(0)
                    emit_av(j0, a0, not pend)
                # softmax normalization: 1/denom broadcast-multiplied into ctxt
                for a in range(2):
                    # copy denom to SBUF first: reciprocal_approx_fast is a
                    # bitwise trick and must not read the PSUM port directly
                    dn0 = smp.tile([1, 512], f32, tag="dn0", name="dn0")
                    nc.vector.tensor_copy(dn0, cps[a][ds(DH, 1), :])
                    rd = smp.tile([1, 512], f32, tag="rd", name="rd")
                    nc.vector.reciprocal_approx_fast(rd, dn0)
                    bc = smp.tile([DH, 512], f32, tag="bc", name="bc")
                    nc.gpsimd.partition_broadcast(bc, rd, channels=DH)
                    cdst = (ctxt16[ds(a * DH, DH), hp, :] if sb == 0 else
                            ctxt[ds(a * DH, DH), hp, ts(sb, 512)])
                    nc.vector.tensor_mul(cdst, cps[a][0:DH, :], bc)

        # drain remaining fillers + final output projection
        emit_filler(len(filler))
        for st in range(4 * (NSB - 1), 4 * NSB):
            for nb in range(2):
                po_unit(st, nb)

        cx.close()

    nc.finalize()
    _built[key] = nc
    return nc


def _is_causal(masked: np.ndarray) -> bool:
    c = ~np.tril(np.ones((S, S), dtype=bool))
    return all(np.array_equal(masked[b], c) for b in range(masked.shape[0]))


def kernel(Q, K, V, masked, WQ_w, WQ_b, WK_w, WK_b, WV_w, WV_b, Wo_w, Wo_b):
    global LAST_RESULT
    from concourse.bass_utils import run_bass_kernel_spmd

    Q = np.asarray(Q, dtype=np.float32)
    K = np.asarray(K, dtype=np.float32)
    V = np.asarray(V, dtype=np.float32)
    masked = np.asarray(masked)
    causal = _is_causal(masked)
    if not causal:
        return _kernel_legacy(Q, K, V, masked, WQ_w, WQ_b, WK_w, WK_b,
                              WV_w, WV_b, Wo_w, Wo_b)
    nc = _build_v2(MMDT)
    if MMDT == "f16":
        npdt = np.float16
    elif MMDT == "bf16":
        import ml_dtypes
        npdt = ml_dtypes.bfloat16
    else:
        npdt = np.float32

    import ml_dtypes
    f8 = ml_dtypes.float8_e4m3fn
    qT = [np.ascontiguousarray(Q[b].T.astype(f8)) for b in range(B)]
    kT = [np.ascontiguousarray(K[b].T.astype(f8)) for b in range(B)]
    vT16 = [np.ascontiguousarray(V[b, :512].T.astype(npdt)) for b in range(B)]
    vT8 = [np.ascontiguousarray(V[b, 512:].T.astype(f8)) for b in range(B)]

    j = np.arange(128)[:, None]
    c = np.arange(128)[None, :]
    mtri_1 = ((j > c) * np.float32(NEG)).astype(npdt)
    mtri_full = np.ascontiguousarray(
        np.broadcast_to(mtri_1[:, None, :], (128, 2, 128)))

    in_maps = []
    for cc in range(NCORES):
        b = cc // CORES_PER_BATCH
        h0 = (cc % CORES_PER_BATCH) * HPC
        sel = slice(h0 * DH, (h0 + HPC) * DH)
        wo_pad = np.asarray(Wo_w).T[sel].reshape(2, 128, D).astype(np.float32)
        m = {
            "qt": qT[b], "kt": kT[b], "vt16": vT16[b], "vt8": vT8[b],
            "wq": np.ascontiguousarray(np.asarray(WQ_w)[sel].T.astype(f8)),
            "wk": np.ascontiguousarray(np.asarray(WK_w)[sel].T.astype(f8)),
            "wv": np.ascontiguousarray(np.asarray(WV_w)[sel].T.astype(npdt)),
            "wv8": np.ascontiguousarray(np.asarray(WV_w)[sel].T.astype(f8)),
            "wo": wo_pad.astype(f8),
            "wo16": wo_pad.astype(npdt),
            "bq": np.ascontiguousarray(np.asarray(WQ_b)[sel].reshape(2, 128).T.astype(np.float32)),
            "bk": np.ascontiguousarray(np.asarray(WK_b)[sel].reshape(2, 128).T.astype(np.float32)),
            "bv": np.ascontiguousarray(np.asarray(WV_b)[sel].reshape(1, HPC * DH).astype(npdt)),
            "mtri": mtri_full,
        }
        m = {k: np.ascontiguousarray(v) for k, v in m.items()}
        in_maps.append(m)

    res = run_bass_kernel_spmd(nc, in_maps, core_ids=list(range(NCORES)), trace=TRACE)
    LAST_RESULT = res

    acc = np.zeros((B, S, D), np.float64)
    for cc in range(NCORES):
        acc[cc // CORES_PER_BATCH] += res.results[cc]["out"].astype(np.float64)
    acc += np.asarray(Wo_b, dtype=np.float64)[None, None, :]
    return acc.astype(np.float32)


# ---------------------------------------------------------------------------
# legacy non-causal fallback (general mask multiply path)
# ---------------------------------------------------------------------------

def _build_legacy(mmdt: str):
    key = ("legacy", mmdt)
    if key in _built:
        return _built[key]
    import concourse.mybir as mybir
    import concourse.tile as tile
    from concourse import bacc
    from concourse.bass import ts, ds

    f32 = mybir.dt.float32
    DT = {"f32r": mybir.dt.float32r, "f16": mybir.dt.float16,
          "bf16": mybir.dt.bfloat16}[mmdt]
    DTNP = {"f32r": f32, "f16": mybir.dt.float16, "bf16": mybir.dt.bfloat16}[mmdt]
    EXP = mybir.ActivationFunctionType.Exp

    nc = bacc.Bacc("TRN2")
    qt = nc.dram_tensor("qt", [D, S], DTNP, kind="ExternalInput")
    kt = nc.dram_tensor("kt", [D, S], DTNP, kind="ExternalInput")
    vt = nc.dram_tensor("vt", [D, S], DTNP, kind="ExternalInput")
    wq = nc.dram_tensor("wq", [D, HPC * DH], DTNP, kind="ExternalInput")
    wk = nc.dram_tensor("wk", [D, HPC * DH], DTNP, kind="ExternalInput")
    wv = nc.dram_tensor("wv", [D, HPC * DH], DTNP, kind="ExternalInput")
    wo = nc.dram_tensor("wo", [2, 128, D], DTNP, kind="ExternalInput")
    bq = nc.dram_tensor("bq", [128, 2], f32, kind="ExternalInput")
    bk = nc.dram_tensor("bk", [128, 2], f32, kind="ExternalInput")
    bv = nc.dram_tensor("bv", [1, HPC * DH], DTNP, kind="ExternalInput")
    mt = nc.dram_tensor("mt", [S, S], DTNP, kind="ExternalInput")
    out = nc.dram_tensor("out", [S, D], f32, kind="ExternalOutput")

    NSB = S // 512
    NST = S // 128

    import contextlib
    with tile.TileContext(nc) as tc, contextlib.ExitStack() as ctx_pools:
        with (
            tc.tile_pool(name="persist", bufs=1) as pp,
            tc.tile_pool(name="sc_ps", bufs=2, space="PSUM") as sc_ps,
            tc.tile_pool(name="ctx_ps", bufs=1, space="PSUM") as ctx_ps,
            tc.tile_pool(name="mm_ps", bufs=2, space="PSUM") as mm_ps,
        ):
            bq_t = pp.tile([128, 2], f32)
            nc.gpsimd.dma_start(out=bq_t, in_=bq[:, :])
            bk_t = pp.tile([128, 2], f32)
            nc.gpsimd.dma_start(out=bk_t, in_=bk[:, :])
            bv_t = pp.tile([1, HPC * DH], DT)
            nc.gpsimd.dma_start(out=bv_t, in_=bv[:, :].bitcast(DT))

            ones_c = pp.tile([1, 128], DT)
            nc.vector.memset(ones_c.bitcast(DTNP), 1.0)

            ap = ctx_pools.enter_context(tc.tile_pool(name="attn", bufs=3))
            smp = ctx_pools.enter_context(tc.tile_pool(name="small", bufs=3))
            mlp = ctx_pools.enter_context(tc.tile_pool(name="mload", bufs=3))
            owp = ctx_pools.enter_context(tc.tile_pool(name="outw", bufs=1))
            op = ctx_pools.enter_context(tc.tile_pool(name="outp", bufs=4))

            q2t = [pp.tile([128, S], DT, tag=f"q2t{i}", name=f"q2t{i}") for i in range(2)]
            k2t = [pp.tile([128, S], DT, tag=f"k2t{i}", name=f"k2t{i}") for i in range(2)]
            vaug = pp.tile([128, NST, HPC * (DH + 1)], DT)
            for h in range(HPC):
                nc.vector.memset(vaug[:, :, ds(h * 65 + 64, 1)].bitcast(DTNP), 1.0)
            ctxt = [pp.tile([128, S], DT, tag=f"ctxt{i}", name=f"ctxt{i}") for i in range(2)]

            with (
                tc.tile_pool(name="wproj", bufs=1) as wp,
                tc.tile_pool(name="stream", bufs=2) as sp,
            ):
                wq_t = wp.tile([128, 8, 256], DT)
                nc.gpsimd.dma_start(out=wq_t, in_=wq.rearrange("(c p) m -> p c m", p=128).bitcast(DT))
                wk_t = wp.tile([128, 8, 256], DT)
                nc.gpsimd.dma_start(out=wk_t, in_=wk.rearrange("(c p) m -> p c m", p=128).bitcast(DT))
                wv_t = wp.tile([128, 8, 256], DT)
                nc.gpsimd.dma_start(out=wv_t, in_=wv.rearrange("(c p) m -> p c m", p=128).bitcast(DT))

                for sb in range(NSB):
                    qs = [sp.tile([128, 4, 512], DT, tag=f"qs{i}", name=f"qs{i}", bufs=1) for i in range(2)]
                    ks = [sp.tile([128, 4, 512], DT, tag=f"ks{i}", name=f"ks{i}", bufs=1) for i in range(2)]
                    vs = [sp.tile([128, 4, 512], DT, tag=f"vs{i}", name=f"vs{i}", bufs=1) for i in range(2)]
                    for half in range(2):
                        for name, t, dr, eng in (("q", qs, qt, nc.sync),
                                                 ("k", ks, kt, nc.sync),
                                                 ("v", vs, vt, nc.gpsimd)):
                            src = dr.rearrange("(c p) s -> p c s", p=128)
                            eng.dma_start(
                                out=t[half],
                                in_=src[:, ds(half * 4, 4), ts(sb, 512)].bitcast(DT),
                            )
                    for hp in range(2):
                        psq = mm_ps.tile([128, 512], f32, tag="mm")
                        for c in range(8):
                            nc.tensor.matmul(
                                psq, wq_t[:, c, ts(hp, 128)], qs[c // 4][:, c % 4, :],
                                start=(c == 0), stop=(c == 7),
                            )
                        nc.vector.tensor_scalar_add(
                            q2t[hp][:, ts(sb, 512)], psq, bq_t[:, ds(hp, 1)])
                        psk = mm_ps.tile([128, 512], f32, tag="mm")
                        for c in range(8):
                            nc.tensor.matmul(
                                psk, wk_t[:, c, ts(hp, 128)], ks[c // 4][:, c % 4, :],
                                start=(c == 0), stop=(c == 7),
                            )
                        nc.vector.tensor_scalar_add(
                            k2t[hp][:, ts(sb, 512)], psk, bk_t[:, ds(hp, 1)])
                    for st4 in range(4):
                        st = sb * 4 + st4
                        psv = mm_ps.tile([128, 256], f32, tag="mm")
                        for c in range(8):
                            nc.tensor.matmul(
                                psv, vs[c // 4][:, c % 4, ts(st4, 128)], wv_t[:, c, :],
                                start=(c == 0), stop=False,
                            )
                        nc.tensor.matmul(psv, ones_c, bv_t, start=False, stop=True)
                        nc.vector.tensor_copy(
                            vaug[:, st, :].rearrange("p (h x) -> p h x", h=HPC)[:, :, 0:DH],
                            psv.rearrange("p (h x) -> p h x", h=HPC),
                        )

            wo_t = owp.tile([128, 2, D], DT)
            nc.gpsimd.dma_start(out=wo_t, in_=wo.rearrange("h p n -> p h n").bitcast(DT))
            outq = []

            def emit_out_unit():
                if not outq:
                    return
                st, nb = outq.pop(0)
                po = mm_ps.tile([128, 512], f32, tag="mm", name="po")
                for k in range(2):
                    nc.tensor.matmul(
                        po, ctxt[k][:, ts(st, 128)],
                        wo_t[:, k, ts(nb, 512)],
                        start=(k == 0), stop=(k == 1),
                    )
                ot = op.tile([128, 512], f32, tag="ot", name="ot")
                nc.any.tensor_copy(ot, po)
                (nc.gpsimd if (st + nb) % 2 else nc.sync).dma_start(out=out[ts(st, 128), ts(nb, 512)], in_=ot)

            for sb in range(NSB):
                for hp in range(2):
                    jts = list(range(NST))
                    cps = [ctx_ps.tile([DH + 1, 512], f32, tag=f"ctx{a}", name=f"cps{a}")
                           for a in range(2)]
                    pending = None

                    def emit_attnv(pjt, pat, last):
                        for a in range(2):
                            h = 2 * hp + a
                            nc.tensor.matmul(
                                cps[a],
                                vaug[:, pjt, ds(h * 65, DH + 1)],
                                pat[:, ds(a * 512, 512)],
                                start=(pjt == jts[0]), stop=last,
                            )

                    for jt in jts:
                        emit_out_unit()
                        sc = sc_ps.tile([128, 1024], f32, tag="sc")
                        mt_t = mlp.tile([128, 512], DT, tag="mt")
                        nc.sync.dma_start(
                            out=mt_t,
                            in_=mt[ts(jt, 128), ts(sb, 512)].bitcast(DT))
                        for a in range(2):
                            nc.tensor.matmul(
                                sc[:, ds(a * 512, 512)],
                                k2t[hp][ds(a * 64, 64), ts(jt, 128)],
                                q2t[hp][ds(a * 64, 64), ds(sb * 512, 512)],
                                start=True, stop=True,
                                tile_position=(a * 64, 0),
                            )
                        at = ap.tile([128, 1024], DT, tag="at")
                        nc.scalar.activation(at, sc, EXP, scale=0.125)
                        for a in range(2):
                            nc.vector.tensor_mul(
                                at[:, ts(a, 512)], at[:, ts(a, 512)], mt_t)
                        if pending is not None:
                            emit_attnv(pending[0], pending[1], False)
                        pending = (jt, at)
                    emit_attnv(pending[0], pending[1], True)
                    for a in range(2):
                        dn0 = smp.tile([1, 512], f32, tag="dn0", name="dn0")
                        nc.vector.tensor_copy(dn0, cps[a][ds(DH, 1), :])
                        rd = smp.tile([1, 512], f32, tag="rd", name="rd")
                        nc.vector.reciprocal_approx_fast(rd, dn0)
                        bc = smp.tile([DH, 512], f32, tag="bc", name="bc")
                        nc.gpsimd.partition_broadcast(bc, rd, channels=DH)
                        nc.vector.tensor_mul(
                            ctxt[hp][ds(a * DH, DH), ts(sb, 512)],
                            cps[a][0:DH, :], bc)
                outq.extend((st, nb) for st in range(4 * sb, 4 * sb + 4)
                            for nb in range(2))
            while outq:
                emit_out_unit()

            ctx_pools.close()

    nc.finalize()
    _built[key] = nc
    return nc


def _kernel_legacy(Q, K, V, masked, WQ_w, WQ_b, WK_w, WK_b, WV_w, WV_b, Wo_w, Wo_b):
    global LAST_RESULT
    from concourse.bass_utils import run_bass_kernel_spmd

    nc = _build_legacy(MMDT)
    if MMDT == "f16":
        npdt = np.float16
    elif MMDT == "bf16":
        import ml_dtypes
        npdt = ml_dtypes.bfloat16
    else:
        npdt = np.float32

    qT = [np.ascontiguousarray(Q[b].T.astype(npdt)) for b in range(B)]
    kT = [np.ascontiguousarray(K[b].T.astype(npdt)) for b in range(B)]
    vT = [np.ascontiguousarray(V[b].T.astype(npdt)) for b in range(B)]
    mtb = [np.ascontiguousarray(
        np.where(masked[b].T, np.float32(0.0), np.float32(1.0)).astype(npdt))
        for b in range(B)]

    in_maps = []
    for c in range(NCORES):
        b = c // CORES_PER_BATCH
        h0 = (c % CORES_PER_BATCH) * HPC
        sel = slice(h0 * DH, (h0 + HPC) * DH)
        wo_pad = np.asarray(Wo_w).T[sel].reshape(2, 128, D).astype(np.float32)
        m = {
            "qt": qT[b], "kt": kT[b], "vt": vT[b],
            "wq": np.ascontiguousarray(np.asarray(WQ_w)[sel].T.astype(npdt)),
            "wk": np.ascontiguousarray(np.asarray(WK_w)[sel].T.astype(npdt)),
            "wv": np.ascontiguousarray(np.asarray(WV_w)[sel].T.astype(npdt)),
            "wo": wo_pad.astype(npdt),
            "bq": np.ascontiguousarray(np.asarray(WQ_b)[sel].reshape(2, 128).T.astype(np.float32)),
            "bk": np.ascontiguousarray(np.asarray(WK_b)[sel].reshape(2, 128).T.astype(np.float32)),
            "bv": np.ascontiguousarray(np.asarray(WV_b)[sel].reshape(1, HPC * DH).astype(npdt)),
            "mt": mtb[b],
        }
        m = {k: np.ascontiguousarray(v) for k, v in m.items()}
        in_maps.append(m)

    res = run_bass_kernel_spmd(nc, in_maps, core_ids=list(range(NCORES)), trace=TRACE)
    LAST_RESULT = res

    acc = np.zeros((B, S, D), np.float64)
    for c in range(NCORES):
        acc[c // CORES_PER_BATCH] += res.results[c]["out"].astype(np.float64)
    acc += np.asarray(Wo_b, dtype=np.float64)[None, None, :]
    return acc.astype(np.float32)



# revision 70
# speedup vs baseline: 1.4272x; 1.0051x over previous
"""Multi-head attention (B=2, S=2048, D=1024, H=16) on 8 TRN2 NeuronCores.

Sharding: batch (2) x head-groups (4 heads/core). Each core computes its
batch's QKV projections for its 4 heads, causal attention, and a partial
output projection over its head slice; the host sums the 4 partials per
batch and adds the output bias.

Layout strategy: everything runs in "transposed" orientation so no on-chip
transposes are needed:
  q2^T[dm, s] = Wq[dm,:] @ Q^T       (host supplies Q^T and Wq^T)
  scores^T[j, si] = k^T.T @ q^T      (d_h contraction, 2 heads row-tiled)
  attn^T = exp(scores^T/8 + mask)    (mask added pre-exp via -60000*tri matmul)
  ctx^T+denom = [v | 1].T @ attn^T   (ones column gives softmax denominator)
  out[s, n] = ctxn^T.T @ Wo^T        (K=128 chunks x2)
v2: single interleaved pipeline -- QKV projection and output projection
units are emitted as fillers inside the attention jt loop so the tensor
queue never drains (keeps HAM at full clock); mask applied pre-exp on the
tensor engine; straddle exp in one strided-AP instruction; f16 output.
"""

import numpy as np

B, S, D, H, DH = 2, 2048, 1024, 16, 64
NCORES = 8
CORES_PER_BATCH = 4
HPC = H // CORES_PER_BATCH  # heads per core = 4
NEG = -60000.0  # exp((x+NEG)/8) == 0 exactly in fp32; fits in fp16
MMDT = "f16"

TRACE = False  # test.py sets True to collect an NTFF profile
LAST_RESULT = None  # BassKernelResults from the last run (for test.py)

_built = {}


def _build_v2(mmdt: str):
    key = ("v2", mmdt)
    if key in _built:
        return _built[key]
    import concourse.mybir as mybir
    import concourse.tile as tile
    from concourse import bacc
    from concourse.bass import ts, ds

    f32 = mybir.dt.float32
    DT = {"f32r": mybir.dt.float32r, "f16": mybir.dt.float16,
          "bf16": mybir.dt.bfloat16}[mmdt]
    DTNP = {"f32r": f32, "f16": mybir.dt.float16, "bf16": mybir.dt.bfloat16}[mmdt]
    EXP = mybir.ActivationFunctionType.Exp
    DT8 = mybir.dt.float8e4
    DROW = mybir.MatmulPerfMode.DoubleRow

    nc = bacc.Bacc("TRN2")
    qt = nc.dram_tensor("qt", [D, S], DT8, kind="ExternalInput")
    kt = nc.dram_tensor("kt", [D, S], DT8, kind="ExternalInput")
    # V split by key range: keys < 512 stay f16 (feed the exact early-query
    # path); keys >= 512 are fp8 (only ever read by spread-attention queries)
    vt16 = nc.dram_tensor("vt16", [D, 512], DTNP, kind="ExternalInput")
    vt8 = nc.dram_tensor("vt8", [D, S - 512], DT8, kind="ExternalInput")
    wq = nc.dram_tensor("wq", [D, HPC * DH], DT8, kind="ExternalInput")
    wk = nc.dram_tensor("wk", [D, HPC * DH], DT8, kind="ExternalInput")
    wv = nc.dram_tensor("wv", [D, HPC * DH], DTNP, kind="ExternalInput")
    wv8 = nc.dram_tensor("wv8", [D, HPC * DH], DT8, kind="ExternalInput")
    wo = nc.dram_tensor("wo", [2, 128, D], DT8, kind="ExternalInput")
    wo16 = nc.dram_tensor("wo16", [2, 128, D], DTNP, kind="ExternalInput")
    bq = nc.dram_tensor("bq", [128, 2], f32, kind="ExternalInput")
    bk = nc.dram_tensor("bk", [128, 2], f32, kind="ExternalInput")
    bv = nc.dram_tensor("bv", [1, HPC * DH], DTNP, kind="ExternalInput")
    mtri = nc.dram_tensor("mtri", [128, 2, 128], DTNP, kind="ExternalInput")
    out = nc.dram_tensor("out", [S, D], mybir.dt.float16, kind="ExternalOutput")

    NSB = S // 512   # 4 si-blocks of 512
    NST = S // 128   # 16 s-tiles / j-tiles of 128

    import contextlib
    with tile.TileContext(nc) as tc, contextlib.ExitStack() as cx:
        pp = cx.enter_context(tc.tile_pool(name="persist", bufs=1))
        sc_ps = cx.enter_context(tc.tile_pool(name="sc_ps", bufs=2, space="PSUM"))
        ctx_ps = cx.enter_context(tc.tile_pool(name="ctx_ps", bufs=1, space="PSUM"))
        mm_ps = cx.enter_context(tc.tile_pool(name="mm_ps", bufs=2, space="PSUM"))
        sp = cx.enter_context(tc.tile_pool(name="stream", bufs=1))
        ap = cx.enter_context(tc.tile_pool(name="attn", bufs=4))
        smp = cx.enter_context(tc.tile_pool(name="small", bufs=3))
        op = cx.enter_context(tc.tile_pool(name="outp", bufs=4))

        # ---- persistent tiles + constant DMAs (small, on scalar queue) ----
        bq_t = pp.tile([128, 2], f32)
        nc.scalar.dma_start(out=bq_t, in_=bq[:, :])
        bk_t = pp.tile([128, 2], f32)
        nc.scalar.dma_start(out=bk_t, in_=bk[:, :])
        bv_t = pp.tile([1, HPC * DH], DT)
        nc.scalar.dma_start(out=bv_t, in_=bv[:, :].bitcast(DT))
        mtri_t = pp.tile([128, 2, 128], DT)
        nc.scalar.dma_start(out=mtri_t, in_=mtri[:, :, :].bitcast(DT))

        ones_c = pp.tile([1, 128], DT)
        nc.vector.memset(ones_c.bitcast(DTNP), 1.0)
        warm = pp.tile([128, 512], DT, name="warm")
        nc.vector.memset(warm.bitcast(DTNP), 1.0)

        q2t = [pp.tile([128, S], DT, tag=f"q2t{i}", name=f"q2t{i}") for i in range(2)]
        k2t = [pp.tile([128, S], DT, tag=f"k2t{i}", name=f"k2t{i}") for i in range(2)]
        # fp8 vaug in 96-wide head blocks (v 0:64 | ones 64 | zero pad 65:96):
        # dual-fp8 ldweights requires a 32-multiple stationary free size
        HB = 96
        vaug = pp.tile([128, NST, HPC * HB], DT8)
        for h in range(HPC):
            nc.vector.memset(vaug[:, :, ds(h * HB + 64, 1)], 1.0)
            nc.vector.memset(vaug[:, :, ds(h * HB + 65, HB - 65)], 0.0)
        # f16 copies for the first query block (sb=0): near-one-hot attention
        # there makes ctx ~ a raw V row, so fp8 V/ctx/Wo would be ~4% off
        vaug16 = pp.tile([128, 4, HPC * (DH + 1)], DT, name="vaug16")
        for h in range(HPC):
            nc.vector.memset(vaug16[:, :, ds(h * 65 + 64, 1)].bitcast(DTNP), 1.0)
        ctxt = pp.tile([128, 2, S], DT8, name="ctxt")
        ctxt16 = pp.tile([128, 2, 512], DT, name="ctxt16")

        wq_t = pp.tile([128, 8, 256], DT8)
        wk_t = pp.tile([128, 8, 256], DT8)
        wv_t = pp.tile([128, 8, 256], DT)
        wv_t8 = pp.tile([128, 8, 256], DT8)
        wo_t = pp.tile([128, 2, D], DT8)
        wo_t16 = pp.tile([128, 2, D], DT)

        # ---- stream tiles: [128, 4, 512] halves, triple buffered over sb so
        # the sb+2 prefetch DMA can start before sb's reads finish ----
        qsrc = qt.rearrange("(c p) s -> p c s", p=128)
        ksrc = kt.rearrange("(c p) s -> p c s", p=128)
        vsrc16 = vt16.rearrange("(c p) s -> p c s", p=128)
        vsrc8 = vt8.rearrange("(c p) s -> p c s", p=128)
        sstreams = {}

        SDT = {"q": DT8, "k": DT8, "v": DT}

        def v_stream_tile(sb, half):
            if sb == 0:
                tl = sp.tile([128, 4, 512], DT, tag=f"v16s{half}",
                             name=f"v16s{half}", bufs=1)
                src = vsrc16[:, ds(half * 4, 4), :].bitcast(DT)
            else:
                tl = sp.tile([128, 4, 512], DT8, tag=f"vs{half}",
                             name=f"vs{half}_{sb}", bufs=3)
                src = vsrc8[:, ds(half * 4, 4), ts(sb - 1, 512)]
            return tl, src

        def emit_stream_dma(sb):
            t = {}
            for name, src in (("q", qsrc), ("k", ksrc)):
                for half in range(2):
                    tl = sp.tile([128, 4, 512], SDT[name], tag=f"{name}s{half}",
                                 name=f"{name}s{half}_{sb}", bufs=3)
                    (nc.sync if half == 0 else nc.gpsimd).dma_start(
                        out=tl, in_=src[:, ds(half * 4, 4), ts(sb, 512)].bitcast(SDT[name]))
                    t[(name, half)] = tl
            for half in range(2):
                tl, src = v_stream_tile(sb, half)
                (nc.sync if half == 0 else nc.gpsimd).dma_start(out=tl, in_=src)
                t[("v", half)] = tl
            sstreams[sb] = t

        def chunk(sb, name, c):
            return sstreams[sb][(name, c // 4)][:, c % 4, :]

        # ---- compute units ----
        def chunk2(sb, name, j):
            # adjacent chunk pair (2j, 2j+1) as a [128, 2, 512] AP for DoubleRow
            half, r = divmod(2 * j, 4)
            return sstreams[sb][(name, half)][:, ds(r, 2), :]

        def psq_unit(sb, hp):
            ps = mm_ps.tile([128, 512], f32, tag="mm", name="psq")
            for j in range(4):
                nc.tensor.matmul(ps, wq_t[:, ds(2 * j, 2), ts(hp, 128)],
                                 chunk2(sb, "q", j),
                                 start=(j == 0), stop=(j == 3), perf_mode=DROW)
            nc.vector.tensor_scalar_add(q2t[hp][:, ts(sb, 512)], ps, bq_t[:, ds(hp, 1)])

        def psk_unit(sb, hp):
            ps = mm_ps.tile([128, 512], f32, tag="mm", name="psk")
            for j in range(4):
                nc.tensor.matmul(ps, wk_t[:, ds(2 * j, 2), ts(hp, 128)],
                                 chunk2(sb, "k", j),
                                 start=(j == 0), stop=(j == 3), perf_mode=DROW)
            nc.vector.tensor_scalar_add(k2t[hp][:, ts(sb, 512)], ps, bk_t[:, ds(hp, 1)])

        def psv_unit(sb, st4):
            st = sb * 4 + st4
            ps = mm_ps.tile([128, 256], f32, tag="mm", name="psv")
            if sb == 0:
                for c in range(8):
                    nc.tensor.matmul(ps, chunk(sb, "v", c)[:, ts(st4, 128)],
                                     wv_t[:, c, :], start=(c == 0), stop=(c == 7))
            else:
                for j in range(4):
                    half, r = divmod(2 * j, 4)
                    vpair = sstreams[sb][("v", half)][:, ds(r, 2), ts(st4, 128)]
                    nc.tensor.matmul(ps, vpair, wv_t8[:, ds(2 * j, 2), :],
                                     start=(j == 0), stop=(j == 3), perf_mode=DROW)
            bv3 = bv_bc.rearrange("p (h x) -> p h x", h=HPC)
            nc.vector.tensor_add(
                vaug[:, st, :].rearrange("p (h x) -> p h x", h=HPC)[:, :, 0:DH],
                ps.rearrange("p (h x) -> p h x", h=HPC), bv3)
            if st < 4:
                nc.vector.tensor_add(
                    vaug16[:, st, :].rearrange("p (h x) -> p h x", h=HPC)[:, :, 0:DH],
                    ps.rearrange("p (h x) -> p h x", h=HPC), bv3)

        def po_unit(st, nb):
            po = mm_ps.tile([128, 512], f32, tag="mm", name="po")
            if st < 4:
                for k in range(2):
                    nc.tensor.matmul(po, ctxt16[:, k, ts(st, 128)],
                                     wo_t16[:, k, ts(nb, 512)],
                                     start=(k == 0), stop=(k == 1))
            else:
                nc.tensor.matmul(po, ctxt[:, :, ts(st, 128)], wo_t[:, :, ts(nb, 512)],
                                 start=True, stop=True, perf_mode=DROW)
            ot = op.tile([128, 512], mybir.dt.float16, tag="ot", name="ot")
            nc.vector.tensor_copy(ot, po)
            nc.gpsimd.dma_start(out=out[ts(st, 128), ts(nb, 512)], in_=ot)

        filler = []

        def emit_filler(n):
            for _ in range(n):
                if filler:
                    filler.pop(0)()

        # ---- prologue ----
        # ~5us of dummy matmuls: spans the DMA wait, flips HAM to full clock
        wps = sc_ps.tile([128, 1024], f32, tag="sc", name="wps")
        for _ in range(12):
            nc.tensor.matmul(wps[:, 0:512], warm[:, 0:128], warm,
                             start=True, stop=True)
        # DMA issue order == global service order == first-needed-first:
        # wq, qh0, qh1, wk, kh0, kh1, wv, vh0, vh1, then sb1; wo on the
        # lightly-used scalar queue so it doesn't delay the stream rails
        def _stile(name, half, sb):
            return sp.tile([128, 4, 512], SDT[name], tag=f"{name}s{half}",
                           name=f"{name}s{half}_{sb}", bufs=3)

        srcs = {"q": qsrc, "k": ksrc}
        t0 = {(n, h): _stile(n, h, 0) for n in "qk" for h in range(2)}
        for h in range(2):
            t0[("v", h)] = v_stream_tile(0, h)[0]
        sstreams[0] = t0
        nc.sync.dma_start(out=wq_t, in_=wq.rearrange("(c p) m -> p c m", p=128).bitcast(DT8))
        for i, (n, h) in enumerate((("q", 0), ("q", 1), ("k", 0), ("k", 1),
                                    ("v", 0), ("v", 1))):
            eng = nc.gpsimd if i % 2 == 0 else nc.sync
            if n == "v":
                src = vsrc16[:, ds(h * 4, 4), :].bitcast(DT)
            else:
                src = srcs[n][:, ds(h * 4, 4), ts(0, 512)].bitcast(SDT[n])
            eng.dma_start(out=t0[(n, h)], in_=src)
            if (n, h) == ("q", 1):
                nc.gpsimd.dma_start(
                    out=wk_t, in_=wk.rearrange("(c p) m -> p c m", p=128).bitcast(DT8))
            if (n, h) == ("k", 1):
                nc.sync.dma_start(
                    out=wv_t, in_=wv.rearrange("(c p) m -> p c m", p=128).bitcast(DT))
                nc.sync.dma_start(
                    out=wv_t8, in_=wv8.rearrange("(c p) m -> p c m", p=128))
        emit_stream_dma(1)
        # one-time broadcast of the V bias for the fused psv drain-add
        bv_bc = pp.tile([128, HPC * DH], DT, name="bv_bc")
        nc.gpsimd.partition_broadcast(bv_bc, bv_t, channels=128)
        psq_unit(0, 0)
        psk_unit(0, 0)

        # ---- main interleaved loop ----
        DEPTH = 3  # attnV deferral depth (pairs)
        for sb in range(NSB):
            if sb < NSB - 2:
                emit_stream_dma(sb + 2)
            if sb == 1:
                # wo loads deferred here so they don't delay the early streams
                nc.gpsimd.dma_start(out=wo_t, in_=wo.rearrange("h p n -> p h n"))
                nc.gpsimd.dma_start(
                    out=wo_t16, in_=wo16.rearrange("h p n -> p h n").bitcast(DT))
            pos = [(st, nb) for st in range(4 * (sb - 1), 4 * sb)
                   for nb in range(2)] if sb >= 1 else []
            for hp in range(2):
                # refill filler queue for this hp
                if sb == 0 and hp == 0:
                    filler.extend([lambda t=t: psv_unit(0, t) for t in range(4)])
                    filler.extend([lambda: psq_unit(0, 1), lambda: psk_unit(0, 1)])
                    filler.extend(
                        [lambda: psq_unit(1, 0), lambda: psk_unit(1, 0),
                         lambda: psq_unit(1, 1), lambda: psk_unit(1, 1)])
                elif hp == 0:
                    filler.extend(
                        (lambda st=st, nb=nb: po_unit(st, nb)) for st, nb in pos[:4])
                    if sb < NSB - 1:
                        nsb = sb + 1
                        filler.extend(
                            [lambda s=nsb: psq_unit(s, 0), lambda s=nsb: psk_unit(s, 0),
                             lambda s=nsb: psq_unit(s, 1), lambda s=nsb: psk_unit(s, 1)])
                if hp == 1:
                    filler.extend(
                        (lambda st=st, nb=nb: po_unit(st, nb)) for st, nb in pos[4:])
                    if sb < NSB - 1:
                        nsb = sb + 1
                        filler.extend(
                            lambda s=nsb, t=t: psv_unit(s, t) for t in range(4))

                jts = list(range(4 * sb + 4))
                nj = len(jts)
                cps = [ctx_ps.tile([HB, 512], f32, tag=f"ctx{a}", name=f"cps{a}")
                       for a in range(2)]
                pend = []
                cur = None

                def lo_of(j):
                    return max(0, (j - 4 * sb) * 128)

                def emit_attnv(pj0, pat, last):
                    # fp8 DoubleRow over the jt pair (pj0, pj0+1): 256-deep
                    # key contraction in one pass
                    lo = lo_of(pj0)
                    for a in range(2):
                        h = 2 * hp + a
                        nc.tensor.matmul(
                            cps[a][:, ds(lo, 512 - lo)],
                            vaug[:, ds(pj0, 2), ds(h * HB, HB)],
                            pat[:, :, ds(a * 512 + lo, 512 - lo)],
                            start=(pj0 == 0), stop=last, perf_mode=DROW)

                def emit_attnv16(pjt, pat, last):
                    # f16 single-jt path for the first query block
                    lo = lo_of(pjt)
                    for a in range(2):
                        h = 2 * hp + a
                        nc.tensor.matmul(
                            cps[a][0:DH + 1, ds(lo, 512 - lo)],
                            vaug16[:, pjt, ds(h * 65, DH + 1)],
                            pat[:, ds(a * 512 + lo, 512 - lo)],
                            start=(pjt == 0), stop=last)

                emit_av = emit_attnv16 if sb == 0 else emit_attnv

                for ji, jt in enumerate(jts):
                    if ji < 2:
                        emit_filler(1)
                    else:
                        left = nj - ji
                        n = max(1, (len(filler) + left - 1) // left) if filler else 0
                        emit_filler(n)
                    straddle = jt >= 4 * sb
                    lo = lo_of(jt)
                    sc = sc_ps.tile([128, 1024], f32, tag="sc")
                    for a in range(2):
                        nc.tensor.matmul(
                            sc[:, ds(a * 512 + lo, 512 - lo)],
                            k2t[hp][ds(a * 64, 64), ts(jt, 128)],
                            q2t[hp][ds(a * 64, 64), ds(sb * 512 + lo, 512 - lo)],
                            start=True, stop=True,
                            tile_position=(a * 64, 0))
                    if straddle:
                        # causal mask: add -60000 over the diagonal triangle
                        # (both heads in one strided op)
                        scv = sc.rearrange("p (a n) -> p a n", a=2)[:, :, ds(lo, 128)]
                        nc.vector.tensor_add(scv, scv, mtri_t)
                    if sb == 0:
                        at = ap.tile([128, 1024], DT, tag="at0", name="at0",
                                     bufs=4)
                        if lo == 0:
                            nc.scalar.activation(at, sc, EXP, scale=0.125)
                        else:
                            nc.scalar.activation(
                                at.rearrange("p (a n) -> p a n", a=2)
                                [:, :, ds(lo, 512 - lo)],
                                sc.rearrange("p (a n) -> p a n", a=2)
                                [:, :, ds(lo, 512 - lo)],
                                EXP, scale=0.125)
                        pend.append((jt, at))
                        if len(pend) > DEPTH:
                            j0, a0 = pend.pop(0)
                            emit_av(j0, a0, False)
                        continue
                    m = ji % 2
                    if m == 0:
                        cur = ap.tile([128, 2, 1024], DT8, tag="atf",
                                      name="atf", bufs=4)
                        if straddle:
                            # zero the pair partner's causally-dead columns on
                            # the (idle) pool engine so they contribute nothing
                            # to the DoubleRow attnV contraction
                            z0 = lo  # partner's dead region is [lo, lo+128)
                            for a in range(2):
                                nc.gpsimd.memset(
                                    cur[:, 1, ds(a * 512 + z0, 128)], 0.0)
                    # terminal window: the tensor drain would otherwise wait on
                    # the ACT backlog; emit a few exps as a DVE int8 Schraudolph
                    # whose bits ARE the fp8e4m3 result
                    fastexp = (sb == NSB - 1 and hp == 1 and 2 <= jt <= 7)
                    if fastexp:
                        # int8 convert truncates toward zero (no rounding), so
                        # bias the Schraudolph constant by +0.5
                        nc.vector.tensor_scalar(
                            cur[:, m, :].bitcast(mybir.dt.int8), sc,
                            0.125 * 8.0 / 0.6931471805599453, 56.15,
                            op0=mybir.AluOpType.mult, op1=mybir.AluOpType.add)
                    elif lo == 0:
                        nc.scalar.activation(cur[:, m, :], sc, EXP, scale=0.125)
                    else:
                        nc.scalar.activation(
                            cur[:, m, :].rearrange("p (a n) -> p a n", a=2)
                            [:, :, ds(lo, 512 - lo)],
                            sc.rearrange("p (a n) -> p a n", a=2)
                            [:, :, ds(lo, 512 - lo)],
                            EXP, scale=0.125)
                    if m == 1:
                        pend.append((jt - 1, cur))
                        if len(pend) > DEPTH:
                            j0, a0 = pend.pop(0)
                            emit_av(j0, a0, False)
                while pend:
                    j0, a0 = pend.pop(0)
                    emit_av(j0, a0, not pend)
                # softmax normalization: 1/denom broadcast-multiplied into ctxt
                for a in range(2):
                    # copy denom to SBUF first: reciprocal_approx_fast is a
                    # bitwise trick and must not read the PSUM port directly
                    dn0 = smp.tile([1, 512], f32, tag="dn0", name="dn0")
                    nc.vector.tensor_copy(dn0, cps[a][ds(DH, 1), :])
                    rd = smp.tile([1, 512], f32, tag="rd", name="rd")
                    nc.vector.reciprocal_approx_fast(rd, dn0)
                    bc = smp.tile([DH, 512], f32, tag="bc", name="bc")
                    nc.gpsimd.partition_broadcast(bc, rd, channels=DH)
                    cdst = (ctxt16[ds(a * DH, DH), hp, :] if sb == 0 else
                            ctxt[ds(a * DH, DH), hp, ts(sb, 512)])
                    nc.vector.tensor_mul(cdst, cps[a][0:DH, :], bc)

        # drain remaining fillers + final output projection
        emit_filler(len(filler))
        for st in range(4 * (NSB - 1), 4 * NSB):
            for nb in range(2):
                po_unit(st, nb)

        cx.close()

    nc.finalize()
    _built[key] = nc
    return nc


def _is_causal(masked: np.ndarray) -> bool:
    c = ~np.tril(np.ones((S, S), dtype=bool))
    return all(np.array_equal(masked[b], c) for b in range(masked.shape[0]))


def kernel(Q, K, V, masked, WQ_w, WQ_b, WK_w, WK_b, WV_w, WV_b, Wo_w, Wo_b):
    global LAST_RESULT
    from concourse.bass_utils import run_bass_kernel_spmd

    Q = np.asarray(Q, dtype=np.float32)
    K = np.asarray(K, dtype=np.float32)
    V = np.asarray(V, dtype=np.float32)
    masked = np.asarray(masked)
    causal = _is_causal(masked)
    if not causal:
        return _kernel_legacy(Q, K, V, masked, WQ_w, WQ_b, WK_w, WK_b,
                              WV_w, WV_b, Wo_w, Wo_b)
    nc = _build_v2(MMDT)
    if MMDT == "f16":
        npdt = np.float16
    elif MMDT == "bf16":
        import ml_dtypes
        npdt = ml_dtypes.bfloat16
    else:
        npdt = np.float32

    import ml_dtypes
    f8 = ml_dtypes.float8_e4m3fn
    qT = [np.ascontiguousarray(Q[b].T.astype(f8)) for b in range(B)]
    kT = [np.ascontiguousarray(K[b].T.astype(f8)) for b in range(B)]
    vT16 = [np.ascontiguousarray(V[b, :512].T.astype(npdt)) for b in range(B)]
    vT8 = [np.ascontiguousarray(V[b, 512:].T.astype(f8)) for b in range(B)]

    j = np.arange(128)[:, None]
    c = np.arange(128)[None, :]
    mtri_1 = ((j > c) * np.float32(NEG)).astype(npdt)
    mtri_full = np.ascontiguousarray(
        np.broadcast_to(mtri_1[:, None, :], (128, 2, 128)))

    in_maps = []
    for cc in range(NCORES):
        b = cc // CORES_PER_BATCH
        h0 = (cc % CORES_PER_BATCH) * HPC
        sel = slice(h0 * DH, (h0 + HPC) * DH)
        wo_pad = np.asarray(Wo_w).T[sel].reshape(2, 128, D).astype(np.float32)
        m = {
            "qt": qT[b], "kt": kT[b], "vt16": vT16[b], "vt8": vT8[b],
            "wq": np.ascontiguousarray(np.asarray(WQ_w)[sel].T.astype(f8)),
            "wk": np.ascontiguousarray(np.asarray(WK_w)[sel].T.astype(f8)),
            "wv": np.ascontiguousarray(np.asarray(WV_w)[sel].T.astype(npdt)),
            "wv8": np.ascontiguousarray(np.asarray(WV_w)[sel].T.astype(f8)),
            "wo": wo_pad.astype(f8),
            "wo16": wo_pad.astype(npdt),
            "bq": np.ascontiguousarray(np.asarray(WQ_b)[sel].reshape(2, 128).T.astype(np.float32)),
            "bk": np.ascontiguousarray(np.asarray(WK_b)[sel].reshape(2, 128).T.astype(np.float32)),
            "bv": np.ascontiguousarray(np.asarray(WV_b)[sel].reshape(1, HPC * DH).astype(npdt)),
            "mtri": mtri_full,
        }
        m = {k: np.ascontiguousarray(v) for k, v in m.items()}
        in_maps.append(m)

    res = run_bass_kernel_spmd(nc, in_maps, core_ids=list(range(NCORES)), trace=TRACE)
    LAST_RESULT = res

    acc = np.zeros((B, S, D), np.float64)
    for cc in range(NCORES):
        acc[cc // CORES_PER_BATCH] += res.results[cc]["out"].astype(np.float64)
    acc += np.asarray(Wo_b, dtype=np.float64)[None, None, :]
    return acc.astype(np.float32)


# ---------------------------------------------------------------------------
# legacy non-causal fallback (general mask multiply path)
# ---------------------------------------------------------------------------

def _build_legacy(mmdt: str):
    key = ("legacy", mmdt)
    if key in _built:
        return _built[key]
    import concourse.mybir as mybir
    import concourse.tile as tile
    from concourse import bacc
    from concourse.bass import ts, ds

    f32 = mybir.dt.float32
    DT = {"f32r": mybir.dt.float32r, "f16": mybir.dt.float16,
          "bf16": mybir.dt.bfloat16}[mmdt]
    DTNP = {"f32r": f32, "f16": mybir.dt.float16, "bf16": mybir.dt.bfloat16}[mmdt]
    EXP = mybir.ActivationFunctionType.Exp

    nc = bacc.Bacc("TRN2")
    qt = nc.dram_tensor("qt", [D, S], DTNP, kind="ExternalInput")
    kt = nc.dram_tensor("kt", [D, S], DTNP, kind="ExternalInput")
    vt = nc.dram_tensor("vt", [D, S], DTNP, kind="ExternalInput")
    wq = nc.dram_tensor("wq", [D, HPC * DH], DTNP, kind="ExternalInput")
    wk = nc.dram_tensor("wk", [D, HPC * DH], DTNP, kind="ExternalInput")
    wv = nc.dram_tensor("wv", [D, HPC * DH], DTNP, kind="ExternalInput")
    wo = nc.dram_tensor("wo", [2, 128, D], DTNP, kind="ExternalInput")
    bq = nc.dram_tensor("bq", [128, 2], f32, kind="ExternalInput")
    bk = nc.dram_tensor("bk", [128, 2], f32, kind="ExternalInput")
    bv = nc.dram_tensor("bv", [1, HPC * DH], DTNP, kind="ExternalInput")
    mt = nc.dram_tensor("mt", [S, S], DTNP, kind="ExternalInput")
    out = nc.dram_tensor("out", [S, D], f32, kind="ExternalOutput")

    NSB = S // 512
    NST = S // 128

    import contextlib
    with tile.TileContext(nc) as tc, contextlib.ExitStack() as ctx_pools:
        with (
            tc.tile_pool(name="persist", bufs=1) as pp,
            tc.tile_pool(name="sc_ps", bufs=2, space="PSUM") as sc_ps,
            tc.tile_pool(name="ctx_ps", bufs=1, space="PSUM") as ctx_ps,
            tc.tile_pool(name="mm_ps", bufs=2, space="PSUM") as mm_ps,
        ):
            bq_t = pp.tile([128, 2], f32)
            nc.gpsimd.dma_start(out=bq_t, in_=bq[:, :])
            bk_t = pp.tile([128, 2], f32)
            nc.gpsimd.dma_start(out=bk_t, in_=bk[:, :])
            bv_t = pp.tile([1, HPC * DH], DT)
            nc.gpsimd.dma_start(out=bv_t, in_=bv[:, :].bitcast(DT))

            ones_c = pp.tile([1, 128], DT)
            nc.vector.memset(ones_c.bitcast(DTNP), 1.0)

            ap = ctx_pools.enter_context(tc.tile_pool(name="attn", bufs=3))
            smp = ctx_pools.enter_context(tc.tile_pool(name="small", bufs=3))
            mlp = ctx_pools.enter_context(tc.tile_pool(name="mload", bufs=3))
            owp = ctx_pools.enter_context(tc.tile_pool(name="outw", bufs=1))
            op = ctx_pools.enter_context(tc.tile_pool(name="outp", bufs=4))

            q2t = [pp.tile([128, S], DT, tag=f"q2t{i}", name=f"q2t{i}") for i in range(2)]
            k2t = [pp.tile([128, S], DT, tag=f"k2t{i}", name=f"k2t{i}") for i in range(2)]
            vaug = pp.tile([128, NST, HPC * (DH + 1)], DT)
            for h in range(HPC):
                nc.vector.memset(vaug[:, :, ds(h * 65 + 64, 1)].bitcast(DTNP), 1.0)
            ctxt = [pp.tile([128, S], DT, tag=f"ctxt{i}", name=f"ctxt{i}") for i in range(2)]

            with (
                tc.tile_pool(name="wproj", bufs=1) as wp,
                tc.tile_pool(name="stream", bufs=2) as sp,
            ):
                wq_t = wp.tile([128, 8, 256], DT)
                nc.gpsimd.dma_start(out=wq_t, in_=wq.rearrange("(c p) m -> p c m", p=128).bitcast(DT))
                wk_t = wp.tile([128, 8, 256], DT)
                nc.gpsimd.dma_start(out=wk_t, in_=wk.rearrange("(c p) m -> p c m", p=128).bitcast(DT))
                wv_t = wp.tile([128, 8, 256], DT)
                nc.gpsimd.dma_start(out=wv_t, in_=wv.rearrange("(c p) m -> p c m", p=128).bitcast(DT))

                for sb in range(NSB):
                    qs = [sp.tile([128, 4, 512], DT, tag=f"qs{i}", name=f"qs{i}", bufs=1) for i in range(2)]
                    ks = [sp.tile([128, 4, 512], DT, tag=f"ks{i}", name=f"ks{i}", bufs=1) for i in range(2)]
                    vs = [sp.tile([128, 4, 512], DT, tag=f"vs{i}", name=f"vs{i}", bufs=1) for i in range(2)]
                    for half in range(2):
                        for name, t, dr, eng in (("q", qs, qt, nc.sync),
                                                 ("k", ks, kt, nc.sync),
                                                 ("v", vs, vt, nc.gpsimd)):
                            src = dr.rearrange("(c p) s -> p c s", p=128)
                            eng.dma_start(
                                out=t[half],
                                in_=src[:, ds(half * 4, 4), ts(sb, 512)].bitcast(DT),
                            )
                    for hp in range(2):
                        psq = mm_ps.tile([128, 512], f32, tag="mm")
                        for c in range(8):
                            nc.tensor.matmul(
                                psq, wq_t[:, c, ts(hp, 128)], qs[c // 4][:, c % 4, :],
                                start=(c == 0), stop=(c == 7),
                            )
                        nc.vector.tensor_scalar_add(
                            q2t[hp][:, ts(sb, 512)], psq, bq_t[:, ds(hp, 1)])
                        psk = mm_ps.tile([128, 512], f32, tag="mm")
                        for c in range(8):
                            nc.tensor.matmul(
                                psk, wk_t[:, c, ts(hp, 128)], ks[c // 4][:, c % 4, :],
                                start=(c == 0), stop=(c == 7),
                            )
                        nc.vector.tensor_scalar_add(
                            k2t[hp][:, ts(sb, 512)], psk, bk_t[:, ds(hp, 1)])
                    for st4 in range(4):
                        st = sb * 4 + st4
                        psv = mm_ps.tile([128, 256], f32, tag="mm")
                        for c in range(8):
                            nc.tensor.matmul(
                                psv, vs[c // 4][:, c % 4, ts(st4, 128)], wv_t[:, c, :],
                                start=(c == 0), stop=False,
                            )
                        nc.tensor.matmul(psv, ones_c, bv_t, start=False, stop=True)
                        nc.vector.tensor_copy(
                            vaug[:, st, :].rearrange("p (h x) -> p h x", h=HPC)[:, :, 0:DH],
                            psv.rearrange("p (h x) -> p h x", h=HPC),
                        )

            wo_t = owp.tile([128, 2, D], DT)
            nc.gpsimd.dma_start(out=wo_t, in_=wo.rearrange("h p n -> p h n").bitcast(DT))
            outq = []

            def emit_out_unit():
                if not outq:
                    return
                st, nb = outq.pop(0)
                po = mm_ps.tile([128, 512], f32, tag="mm", name="po")
                for k in range(2):
                    nc.tensor.matmul(
                        po, ctxt[k][:, ts(st, 128)],
                        wo_t[:, k, ts(nb, 512)],
                        start=(k == 0), stop=(k == 1),
                    )
                ot = op.tile([128, 512], f32, tag="ot", name="ot")
                nc.any.tensor_copy(ot, po)
                (nc.gpsimd if (st + nb) % 2 else nc.sync).dma_start(out=out[ts(st, 128), ts(nb, 512)], in_=ot)

            for sb in range(NSB):
                for hp in range(2):
                    jts = list(range(NST))
                    cps = [ctx_ps.tile([DH + 1, 512], f32, tag=f"ctx{a}", name=f"cps{a}")
                           for a in range(2)]
                    pending = None

                    def emit_attnv(pjt, pat, last):
                        for a in range(2):
                            h = 2 * hp + a
                            nc.tensor.matmul(
                                cps[a],
                                vaug[:, pjt, ds(h * 65, DH + 1)],
                                pat[:, ds(a * 512, 512)],
                                start=(pjt == jts[0]), stop=last,
                            )

                    for jt in jts:
                        emit_out_unit()
                        sc = sc_ps.tile([128, 1024], f32, tag="sc")
                        mt_t = mlp.tile([128, 512], DT, tag="mt")
                        nc.sync.dma_start(
                            out=mt_t,
                            in_=mt[ts(jt, 128), ts(sb, 512)].bitcast(DT))
                        for a in range(2):
                            nc.tensor.matmul(
                                sc[:, ds(a * 512, 512)],
                                k2t[hp][ds(a * 64, 64), ts(jt, 128)],
                                q2t[hp][ds(a * 64, 64), ds(sb * 512, 512)],
                                start=True, stop=True,
                                tile_position=(a * 64, 0),
                            )
                        at = ap.tile([128, 1024], DT, tag="at")
                        nc.scalar.activation(at, sc, EXP, scale=0.125)
                        for a in range(2):
                            nc.vector.tensor_mul(
                                at[:, ts(a, 512)], at[:, ts(a, 512)], mt_t)
                        if pending is not None:
                            emit_attnv(pending[0], pending[1], False)
                        pending = (jt, at)
                    emit_attnv(pending[0], pending[1], True)
                    for a in range(2):
                        dn0 = smp.tile([1, 512], f32, tag="dn0", name="dn0")
                        nc.vector.tensor_copy(dn0, cps[a][ds(DH, 1), :])
                        rd = smp.tile([1, 512], f32, tag="rd", name="rd")
                        nc.vector.reciprocal_approx_fast(rd, dn0)
                        bc = smp.tile([DH, 512], f32, tag="bc", name="bc")
                        nc.gpsimd.partition_broadcast(bc, rd, channels=DH)
                        nc.vector.tensor_mul(
                            ctxt[hp][ds(a * DH, DH), ts(sb, 512)],
                            cps[a][0:DH, :], bc)
                outq.extend((st, nb) for st in range(4 * sb, 4 * sb + 4)
                            for nb in range(2))
            while outq:
                emit_out_unit()

            ctx_pools.close()

    nc.finalize()
    _built[key] = nc
    return nc


def _kernel_legacy(Q, K, V, masked, WQ_w, WQ_b, WK_w, WK_b, WV_w, WV_b, Wo_w, Wo_b):
    global LAST_RESULT
    from concourse.bass_utils import run_bass_kernel_spmd

    nc = _build_legacy(MMDT)
    if MMDT == "f16":
        npdt = np.float16
    elif MMDT == "bf16":
        import ml_dtypes
        npdt = ml_dtypes.bfloat16
    else:
        npdt = np.float32

    qT = [np.ascontiguousarray(Q[b].T.astype(npdt)) for b in range(B)]
    kT = [np.ascontiguousarray(K[b].T.astype(npdt)) for b in range(B)]
    vT = [np.ascontiguousarray(V[b].T.astype(npdt)) for b in range(B)]
    mtb = [np.ascontiguousarray(
        np.where(masked[b].T, np.float32(0.0), np.float32(1.0)).astype(npdt))
        for b in range(B)]

    in_maps = []
    for c in range(NCORES):
        b = c // CORES_PER_BATCH
        h0 = (c % CORES_PER_BATCH) * HPC
        sel = slice(h0 * DH, (h0 + HPC) * DH)
        wo_pad = np.asarray(Wo_w).T[sel].reshape(2, 128, D).astype(np.float32)
        m = {
            "qt": qT[b], "kt": kT[b], "vt": vT[b],
            "wq": np.ascontiguousarray(np.asarray(WQ_w)[sel].T.astype(npdt)),
            "wk": np.ascontiguousarray(np.asarray(WK_w)[sel].T.astype(npdt)),
            "wv": np.ascontiguousarray(np.asarray(WV_w)[sel].T.astype(npdt)),
            "wo": wo_pad.astype(npdt),
            "bq": np.ascontiguousarray(np.asarray(WQ_b)[sel].reshape(2, 128).T.astype(np.float32)),
            "bk": np.ascontiguousarray(np.asarray(WK_b)[sel].reshape(2, 128).T.astype(np.float32)),
            "bv": np.ascontiguousarray(np.asarray(WV_b)[sel].reshape(1, HPC * DH).astype(npdt)),
            "mt": mtb[b],
        }
        m = {k: np.ascontiguousarray(v) for k, v in m.items()}
        in_maps.append(m)

    res = run_bass_kernel_spmd(nc, in_maps, core_ids=list(range(NCORES)), trace=TRACE)
    LAST_RESULT = res

    acc = np.zeros((B, S, D), np.float64)
    for c in range(NCORES):
        acc[c // CORES_PER_BATCH] += res.results[c]["out"].astype(np.float64)
    acc += np.asarray(Wo_b, dtype=np.float64)[None, None, :]
    return acc.astype(np.float32)

